# revision 45
# baseline (speedup 1.0000x reference)
"""AtlasV4Transformer Trainium2 kernel — 8-core SPMD, token-split data parallel.

Sharding: core c -> batch b = c//2, token half = c%2 (450 of 900 grid tokens).
Activations are feature-major on chip: x^T [D(partitions, 3 chunk tiles), tokens].
Attention uses transposed scores S^T[k,q] per head; softmax row sums ride along
the AV matmul via a constant-1 slot built into the head-padded V layout (head h
occupies a 64-wide slot: [1 | v(40) | 0]).  Head pairs are processed jointly:
the two score matmuls of a pair row-pack the PE array (contraction rows 0:40 /
64:104) into one 2-bank PSUM tile, bias-add runs as one DVE pass and exp as one
ACT pass over [kc, 900].  The geometric-transform AllReduce feeds a rank-17
bias column (cvec) folded into tn1's gelu bias, so the tn MLP matmuls never
wait on the collective.  The per-head geometric bias table gather is
materialized on the host and streamed as a pre-interleaved fp8 input.
"""
import sys

import numpy as np

if "/opt/trn_rl_repo" not in sys.path:
    sys.path.insert(0, "/opt/trn_rl_repo")

import concourse.bass as bass
import concourse.bacc as bacc
import concourse.mybir as mybir
from concourse import tile

F32 = mybir.dt.float32
BF16 = mybir.dt.bfloat16
F8 = mybir.dt.float8e4
BIAS_SCALE = 64.0
AF = mybir.ActivationFunctionType
OP = mybir.AluOpType

B, G, D, L, NH, DK, S = 4, 30, 320, 4, 8, 40, 900
SH = S // 2            # tokens owned per core
FFD = 4 * D            # 1280
HP = 512               # head-padded q/k/v width (8 heads x 64)
SCALE = 1.0 / np.sqrt(DK)
EPS = 1e-5
PAIRS = [[0, 1], [2, 3], [4, 5], [6, 7]]
PG = G + 6             # padded grid 36
NTAPG = 42             # conv taps packed 2 per group (84 total)


def chunks(n, c=128):
    return [(i, min(i + c, n)) for i in range(0, n, c)]


DCH = chunks(D)          # 3 feature chunks
# key-token chunks aligned to the 450/450 own|peer split
KCH = chunks(SH) + [(SH + a, SH + b) for (a, b) in chunks(SH)]

# packed per-layer weight blocks: fixed column order shared by host and device
TB1 = [(0, 128), (128, 256), (256, 320)]
WPK_A = [("wq", D, HP, None), ("wk", D, HP, None), ("wv", D, HP, None), ("wo", HP, D, None)]
WPK_M = [("wcat", D, 17, None), ("tw1", D, 640, TB1), ("tw1t", 17, 640, [(0, 17)]),
         ("tw2", 640, D, None), ("tw3", D, D, None), ("fw1", D, FFD, None), ("fw2", FFD, D, None)]


def build_wpk_colmap(specs):
    cm, col = {}, 0
    for name, kdim, ndim, bnd in specs:
        for ci, _ in enumerate(bnd or chunks(kdim)):
            cm[(name, ci)] = col
            col += ndim
    return cm, col


WPKA_CM, WPKA_COLS = build_wpk_colmap(WPK_A)
WPKM_CM, WPKM_COLS = build_wpk_colmap(WPK_M)


def wpk_slices(t, specs, cm):
    out = {}
    for name, kdim, ndim, bnd in specs:
        out[name] = [t[:, cm[(name, ci)]:cm[(name, ci)] + ndim]
                     for ci, _ in enumerate(bnd or chunks(kdim))]
    return out


# packed per-partition vectors: fixed column order shared by host and device
VEC_LAYER_SPECS = [("bqs", HP), ("bk", HP), ("bo", D), ("lag", D), ("lab", D),
                   ("l2g", D), ("l2b", D), ("tb1", 640), ("tb2", D), ("tb3", D),
                   ("fb1", FFD), ("fb2", D)]
VEC_GLOBAL_SPECS = [("inb", D), ("cb0", D), ("cb1", D), ("cb2", D), ("cb3", D),
                    ("fusb", D), ("ob1", 160), ("ob2", 80), ("ob3", 10)]


def build_vec_colmap():
    cm = {}
    col = 0
    for l in range(L):
        for name, n in VEC_LAYER_SPECS:
            for ci in range(len(chunks(n))):
                cm[(name, l, ci)] = col
                col += 1
    for name, n in VEC_GLOBAL_SPECS:
        for ci in range(len(chunks(n))):
            cm[(name, None, ci)] = col
            col += 1
    return cm, col


VEC_COLMAP, VEC_NCOL = build_vec_colmap()


def build(nc):
    dpi = lambda name, shape, dt: nc.declare_dram_parameter(name, list(shape), dt, isOutput=False)

    P = {}
    P["grid"] = dpi("grid", [1, S], BF16)
    P["iota10"] = dpi("iota10", [10, 1], F32)
    P["peT"] = dpi("peT", [D, S], BF16)
    P["inw"] = dpi("inw", [10, D], BF16)
    P["vecpack"] = dpi("vecpack", [128, VEC_NCOL], F32)
    P["wpackA"] = dpi("wpackA", [L, 128, WPKA_COLS], BF16)
    P["wpackM"] = dpi("wpackM", [L, 128, WPKM_COLS], BF16)
    P["bcat"] = dpi("bcat", [L, 1, 17], BF16)
    P["mskp"] = dpi("mskp", [17, 4], BF16)
    P["mskTp"] = dpi("mskTp", [3, 17], BF16)
    # bias, transposed + chunk-padded + host-interleaved: [l, kchunk, r(128), g2, 2*q(450)]
    # (g2 innermost so one contiguous DMA per (kchunk, gp) covers both heads of both j's)
    P["biasT"] = dpi("biasT", [L, len(KCH), 128, NH // 2, 2 * SH], F8)
    P["ck"] = dpi("ck", [NTAPG, D, 2 * D], BF16)
    P["fusw"] = dpi("fusw", [FFD, D], BF16)
    P["ow1"] = dpi("ow1", [D, 160], BF16)
    P["ow2"] = dpi("ow2", [160, 80], BF16)
    P["ow3"] = dpi("ow3", [80, 10], BF16)
    P["out"] = nc.declare_dram_parameter("out", [10, SH], F32, isOutput=True)

    with tile.TileContext(nc) as tc:
        with (
            tc.tile_pool(name="const", bufs=1) as cp,
            tc.tile_pool(name="wts", bufs=1) as wp,
            tc.tile_pool(name="acts", bufs=1) as ap_,
            tc.tile_pool(name="tmp", bufs=1) as tp,
            tc.tile_pool(name="psum", bufs=1, space="PSUM") as pp,
            tc.tile_pool(name="dram", bufs=1, space="DRAM") as dram,
        ):
            build_body(nc, tc, cp, wp, ap_, tp, pp, dram, P)
    return nc


def load_w(nc, wp, param, l, kdim, ndim, name, bufs=1, boundaries=None):
    ts = []
    for ci, (c0, c1) in enumerate(boundaries or chunks(kdim)):
        t = wp.tile([128, ndim], BF16, tag=f"{name}{ci}", bufs=bufs, name=f"{name}{ci}")
        src = param[l, c0:c1, :] if l is not None else param[c0:c1, :]
        nc.sync.dma_start(out=t[: c1 - c0, :], in_=src)
        ts.append(t)
    return ts


def ps_work(pp, name="ps_w"):
    """One 2-bank [128, 1024] f32 PSUM tile from the shared rotation."""
    return pp.tile([128, 1024], F32, tag="ps_w", bufs=3, name=name)


def warm_keep(nc, ones, trash, n):
    """Dependency-free filler matmuls into a throwaway PSUM row.  Emitted into
    known PE-idle windows (collective wait, conv pad construction) so the HAM
    activity monitor keeps the array at the 2.4 GHz clock state."""
    for _ in range(n):
        nc.tensor.matmul(trash, ones[0:128, 0:1], ones[:, 0:SH], start=True, stop=True)


def mm_proj(nc, pp, Wt, X, kdim, ndim, evict):
    kch = chunks(kdim)
    for ni, (n0, n1) in enumerate(chunks(ndim)):
        ps = ps_work(pp, "ps_mm")
        for ci, (c0, c1) in enumerate(kch):
            nc.tensor.matmul(ps[: n1 - n0, 0:SH], Wt[ci][: c1 - c0, n0:n1], X[ci][: c1 - c0, :],
                             start=(ci == 0), stop=(ci == len(kch) - 1))
        evict(ps, ni, n0, n1)


def build_body(nc, tc, cp, wp, ap_, tp, pp, dram, P):
    # ---------------- constants ----------------
    ones = cp.tile([128, SH], BF16, tag="ones", bufs=1, name="ones")
    nc.vector.memset(ones[:], 1.0)
    ones_f = cp.tile([1, 128], F32, tag="ones_f", bufs=1, name="ones_f")
    nc.vector.memset(ones_f[:], 1.0)
    # softmax group masks for the 17-wide geometric transform column (host consts)
    msk = cp.tile([17, 4], BF16, tag="msk", bufs=1, name="msk")
    nc.sync.dma_start(out=msk[:], in_=P["mskp"][:])
    mskT = cp.tile([3, 17], BF16, tag="mskT", bufs=1, name="mskT")
    nc.sync.dma_start(out=mskT[:], in_=P["mskTp"][:])

    eps_t = cp.tile([1, 1], F32, tag="eps", bufs=1, name="eps_t")
    nc.vector.memset(eps_t[:], EPS)
    iota_t = cp.tile([10, 1], F32, tag="iota", bufs=1, name="iota_t")
    nc.sync.dma_start(out=iota_t[:], in_=P["iota10"][:])
    peT_t = [cp.tile([128, S], BF16, tag=f"peT{ci}", bufs=1, name=f"peT{ci}") for ci in range(3)]
    for ci, (c0, c1) in enumerate(DCH):
        nc.sync.dma_start(out=peT_t[ci][: c1 - c0, :], in_=P["peT"][c0:c1, :])
    grid_t = cp.tile([1, S], BF16, tag="grid", bufs=1, name="grid_t")
    nc.sync.dma_start(out=grid_t[:], in_=P["grid"][:])
    inw_t = cp.tile([10, D], BF16, tag="inw", bufs=1, name="inw_t")
    nc.sync.dma_start(out=inw_t[:], in_=P["inw"][:])
    vp = cp.tile([128, VEC_NCOL], F32, tag="vecpack", bufs=1, name="vp")
    nc.sync.dma_start(out=vp[:], in_=P["vecpack"][:])

    def vec_aps(name, n, l=None):
        return [vp[: c1 - c0, VEC_COLMAP[(name, l, ci)]:VEC_COLMAP[(name, l, ci)] + 1]
                for ci, (c0, c1) in enumerate(chunks(n))]

    def vec_wide(name, l, n):
        col0 = VEC_COLMAP[(name, l, 0)]
        return vp[:, col0:col0 + n]

    bcat_t = []
    for l in range(L):
        t2_ = cp.tile([1, 17], BF16, tag=f"bcat{l}", bufs=1, name=f"bcat{l}")
        nc.sync.dma_start(out=t2_[:], in_=P["bcat"][l])
        bcat_t.append(t2_)

    # ---------------- embedding (both halves; kills the layer-0 gather) ----------------
    oh = tp.tile([10, S], BF16, tag="oh", bufs=1, name="oh")
    for half in range(2):
        hs = slice(SH * half, SH * half + SH)
        ps_g = ps_work(pp, "ps_g")
        nc.tensor.matmul(ps_g[:10, 0:SH], ones[0:1, 0:10], grid_t[0:1, hs], start=True, stop=True)
        nc.vector.tensor_scalar(out=oh[:, hs], in0=ps_g[:10, 0:SH], scalar1=iota_t[:10, :],
                                scalar2=None, op0=OP.is_equal)

    inb_c = vec_aps("inb", D)
    xs = [ap_.tile([128, SH], BF16, tag=f"xs{ci}", bufs=1, name=f"xs{ci}") for ci in range(3)]
    xp0 = [ap_.tile([128, SH], BF16, tag=f"xp{ci}", bufs=1, name=f"xp{ci}") for ci in range(3)]
    for ci, (c0, c1) in enumerate(DCH):
        for half in range(2):
            hs = slice(SH * half, SH * half + SH)
            dst = xs[ci] if half == 0 else xp0[ci]
            pse = ps_work(pp, "pse")
            nc.tensor.matmul(pse[: c1 - c0, 0:SH], inw_t[:, c0:c1], oh[:, hs], start=True, stop=True)
            nc.vector.scalar_tensor_tensor(
                out=dst[: c1 - c0, :], in0=pse[: c1 - c0, 0:SH], scalar=inb_c[ci],
                in1=peT_t[ci][: c1 - c0, hs], op0=OP.add, op1=OP.add)
    # ones row for the v-bias contraction trick
    nc.vector.memset(xs[2][64:65, :], 1.0)
    nc.vector.memset(xp0[2][64:65, :], 1.0)

    # ---------------- transformer layers ----------------
    W = load_attn_w(nc, wp, P, 0)
    for l in range(L):
        Wnext = {} if l + 1 < L else None
        xs = layer(nc, wp, ap_, tp, pp, dram, P, l, xs, vec_aps, vec_wide, bcat_t[l],
                   ones, ones_f, msk, mskT, eps_t, xp0 if l == 0 else None, W, Wnext)
        W = Wnext

    # ---------------- conv fusion + head ----------------
    conv_head(nc, cp, wp, ap_, tp, pp, dram, P, xs, vec_aps, ones, ones_f)


def load_attn_w(nc, wp, P, l):
    """q/k/v/o weights as ONE packed DMA, prefetched one layer ahead (bufs=2)."""
    t = wp.tile([128, WPKA_COLS], BF16, tag="wpkA", bufs=2, name="wpkA")
    nc.sync.dma_start(out=t[:], in_=P["wpackA"][l])
    return wpk_slices(t, WPK_A, WPKA_CM)


def layer(nc, wp, ap_, tp, pp, dram, P, l, xs, vec_aps, vec_wide, bcat_t, ones, ones_f,
          msk, mskT, eps_t, xp0, W, Wnext):
    bqs_c = vec_aps("bqs", HP, l)
    bk_c = vec_aps("bk", HP, l)
    bo_c = vec_aps("bo", D, l)
    lag_c = vec_aps("lag", D, l)
    lab_c = vec_aps("lab", D, l)
    l2g_c = vec_aps("l2g", D, l)
    l2b_c = vec_aps("l2b", D, l)
    tb2_c = vec_aps("tb2", D, l)
    tb3_c = vec_aps("tb3", D, l)
    fb1_c = vec_aps("fb1", FFD, l)
    fb2_c = vec_aps("fb2", D, l)
    tb1_w = vec_wide("tb1", l, 5)

    wq_t, wk_t, wv_t, wo_t = W["wq"], W["wk"], W["wv"], W["wo"]
    # MLP weights: one packed DMA issued at layer top, lands under attention
    wm = wp.tile([128, WPKM_COLS], BF16, tag="wpkM", bufs=1, name="wpkM")
    nc.sync.dma_start(out=wm[:], in_=P["wpackM"][l])
    WM = wpk_slices(wm, WPK_M, WPKM_CM)
    wcat_t, tw1_t, tw1t_t, tw2_t = WM["wcat"], WM["tw1"], WM["tw1t"][0], WM["tw2"]
    tw3_t, fw1_t, fw2_t = WM["tw3"], WM["fw1"], WM["fw2"]
    if Wnext is not None:
        Wnext.update(load_attn_w(nc, wp, P, l + 1))

    # ---- q projection, scaled; head-padded rows [64h, 64h+40) ----
    qp = [ap_.tile([128, SH], BF16, tag=f"qp{g}", bufs=1, name=f"qp{g}") for g in range(4)]

    def evict_q(ps, ni, n0, n1):
        nc.vector.tensor_scalar(out=qp[ni][: n1 - n0, :], in0=ps[: n1 - n0, 0:SH],
                                scalar1=SCALE, scalar2=bqs_c[ni], op0=OP.mult, op1=OP.add)

    mm_proj(nc, pp, wq_t, xs, D, HP, evict_q)

    # ---- peer-x gather (single collective; layer 0 has xp precomputed) ----
    if xp0 is not None:
        xp = xp0
    else:
        xgin = dram.tile([D, SH], BF16, tag="xgin", bufs=2, name="xgin")
        xgout = dram.tile([2, D, SH], BF16, tag="xgout", bufs=2, name="xgout")
        for ci, (c0, c1) in enumerate(DCH):
            nc.sync.dma_start(out=xgin[c0:c1, :], in_=xs[ci][: c1 - c0, :])
        nc.gpsimd.collective_compute("AllGather", OP.bypass, replica_groups=PAIRS,
                                     ins=[xgin[:].opt()], outs=[xgout[:].opt()])
        peer = (nc.sync.partition_id() + 1) % 2
        xgout_f = xgout[:].rearrange("g p q -> (g p) q")
        xp = [ap_.tile([128, SH], BF16, tag=f"xp{ci}", bufs=1, name=f"xp{ci}") for ci in range(3)]
        for ci, (c0, c1) in enumerate(DCH):
            nc.sync.dma_start(out=xp[ci][: c1 - c0, :],
                              in_=xgout_f[bass.ds(peer * D + c0, c1 - c0), :])
        nc.vector.memset(xp[2][64:65, :], 1.0)

    # ---- k for all 900 keys, feature-major [own cols | peer cols] ----
    khp = [ap_.tile([128, S], BF16, tag=f"khp{g2}", bufs=1, name=f"khp{g2}") for g2 in range(4)]

    def evict_k_own(ps, ni, n0, n1):
        nc.vector.tensor_scalar(out=khp[ni][: n1 - n0, 0:SH], in0=ps[: n1 - n0, 0:SH],
                                scalar1=bk_c[ni], scalar2=None, op0=OP.add)

    mm_proj(nc, pp, wk_t, xs, D, HP, evict_k_own)

    def evict_k_peer(ps, ni, n0, n1):
        nc.vector.tensor_scalar(out=khp[ni][: n1 - n0, SH:S], in0=ps[: n1 - n0, 0:SH],
                                scalar1=bk_c[ni], scalar2=None, op0=OP.add)

    # ---- v, token-major rows [own | peer], 64-wide head slots; the v bias and
    # the softmax-sum 1-slots ride contraction row 64 of chunk 2 (ones row) ----
    def v_chunks(rng):
        for si in rng:
            k0, k1 = KCH[si]
            kc = k1 - k0
            src_x, off = (xs, 0) if k1 <= SH else (xp, SH)
            psv = ps_work(pp, "psv")
            for ci, (c0, c1) in enumerate(DCH):
                kk = (c1 - c0) + (1 if ci == 2 else 0)
                nc.tensor.matmul(psv[:kc, 0:HP], src_x[ci][:kk, k0 - off:k1 - off],
                                 wv_t[ci][:kk, :], start=(ci == 0), stop=(ci == 2))
            t = ap_.tile([128, HP], BF16, tag=f"va{si}", bufs=1, name=f"va{si}")
            nc.vector.tensor_copy(t[:kc, :], psv[:kc, 0:HP])
            va.append(t)

    va = []
    v_chunks(range(4))
    mm_proj(nc, pp, wk_t, xp, D, HP, evict_k_peer)
    v_chunks(range(4, 8))

    # ---- attention: head pairs processed jointly; all own-key chunks (ci<4)
    # run before any peer-key dependency, hiding the x-gather latency ----
    attnT = [ap_.tile([128, SH], BF16, tag=f"at{g}", bufs=2, name=f"at{g}") for g in range(4)]
    for gp in range(2):
        ps_avs = [pp.tile([128, SH], F32, tag=f"ps_av{j}", bufs=1, name=f"ps_av{j}")
                  for j in range(2)]

        def emit_av(j, ci, kc, ee):
            # col-packed AV pair: the two heads use disjoint 32-col groups
            a0 = 128 * (2 * gp + j)
            nc.tensor.matmul(ps_avs[j][0:64, :], va[ci][:kc, a0:a0 + 64],
                             ee[:kc, 0:SH], start=(ci == 0), stop=(ci == len(KCH) - 1),
                             tile_position=(0, 0))
            nc.tensor.matmul(ps_avs[j][64:128, :], va[ci][:kc, a0 + 64:a0 + 128],
                             ee[:kc, SH:2 * SH], start=(ci == 0), stop=(ci == len(KCH) - 1),
                             tile_position=(0, 64))

        pend = []   # AV pairs lag their scores by 1 unit
        for ci, (k0, k1) in enumerate(KCH):
            kc = k1 - k0
            btp = tp.tile([128, 4 * SH], F8, tag="bias", bufs=2, name="btp")
            nc.gpsimd.dma_start(
                out=btp[:kc, :],
                in_=P["biasT"][l, ci, :kc, 2 * gp:2 * gp + 2, :].rearrange("p g q -> p (g q)"))
            for j in range(2):
                g2 = 2 * gp + j
                bt2 = btp[:, 2 * SH * j:2 * SH * j + 2 * SH]
                psp = ps_work(pp, "psp")
                # row-packed score pair: contraction rows 0:40 and 64:104 run
                # concurrently in disjoint 32-row groups of the PE array
                nc.tensor.matmul(psp[:kc, 0:SH], khp[g2][0:40, k0:k1], qp[g2][0:40, :],
                                 start=True, stop=True, tile_position=(0, 0))
                nc.tensor.matmul(psp[:kc, 512:512 + SH], khp[g2][64:104, k0:k1],
                                 qp[g2][64:104, :], start=True, stop=True,
                                 tile_position=(64, 0))
                es = tp.tile([128, 2 * SH], BF16, tag="esc", bufs=2, name="es")
                nc.vector.scalar_tensor_tensor(
                    out=es[:kc].rearrange("p (h q) -> p h q", h=2),
                    in0=bt2[:kc].rearrange("p (h q) -> p h q", h=2),
                    scalar=1.0 / BIAS_SCALE,
                    in1=psp[:kc].rearrange("p (h q) -> p h q", h=2)[:, :, 0:SH],
                    op0=OP.mult, op1=OP.add)
                ee = tp.tile([128, 2 * SH], BF16, tag="eexp", bufs=2, name="ee")
                nc.scalar.activation(ee[:kc, :], es[:kc, :], AF.Exp)
                pend.append((j, ci, kc, ee))
                if len(pend) > 1:
                    emit_av(*pend.pop(0))
        for u in pend:
            emit_av(*u)
        for j in range(2):
            g2 = 2 * gp + j
            ps_av = ps_avs[j]
            # sum rows 0 / 64 -> SBUF rows, broadcast to partition halves via
            # matmul, then reciprocal runs partition-parallel on [128, SH]
            s2a = tp.tile([1, SH], BF16, tag="rec", bufs=2, name="s2a")
            s2b = tp.tile([1, SH], BF16, tag="recb", bufs=2, name="s2b")
            nc.vector.tensor_copy(s2a[:], ps_av[0:1, :])
            nc.vector.tensor_copy(s2b[:], ps_av[64:65, :])
            ps_bc = ps_work(pp, "ps_bc")
            nc.tensor.matmul(ps_bc[0:64, 0:SH], ones[0:1, 0:64], s2a[:], start=True, stop=True)
            nc.tensor.matmul(ps_bc[64:128, 0:SH], ones[0:1, 0:64], s2b[:], start=True, stop=True)
            bc = tp.tile([128, SH], F32, tag="bcn", bufs=1, name="bc")
            nc.vector.reciprocal_approx_fast(bc[:], ps_bc[:, 0:SH])
            nc.vector.tensor_tensor(out=attnT[g2][:], in0=ps_av[:], in1=bc[:], op=OP.mult)

    # ---- wo projection + residual + LN ----
    res = [tp.tile([128, SH], BF16, tag=f"res{ci}", bufs=2, name=f"res{ci}") for ci in range(3)]

    def evict_o(ps, ni, n0, n1):
        nc.vector.scalar_tensor_tensor(out=res[ni][: n1 - n0, :], in0=ps[: n1 - n0, 0:SH],
                                       scalar=bo_c[ni], in1=xs[ni][: n1 - n0, :],
                                       op0=OP.add, op1=OP.add)

    mm_proj(nc, pp, wo_t, attnT, HP, D, evict_o)
    xs1 = layernorm(nc, ap_, tp, pp, res, lag_c, lab_c, ones, ones_f, eps_t, "xsa")

    # ---- geometric transform: pair all-reduce -> 17-wide transform params ->
    # rank-17 contribution becomes a per-partition bias column for tn1 ----
    gin = dram.tile([128, 3], F32, tag="gin", bufs=2, name="gin")
    gout = dram.tile([128, 3], F32, tag="gout", bufs=2, name="gout")
    gred = tp.tile([128, 3], F32, tag="gred", bufs=2, name="gred")
    for ci, (c0, c1) in enumerate(DCH):
        nc.vector.reduce_sum(gred[: c1 - c0, ci:ci + 1], xs1[ci][: c1 - c0, :],
                             axis=mybir.AxisListType.X)
    nc.sync.dma_start(out=gin[:], in_=gred[:])
    nc.gpsimd.collective_compute("AllReduce", OP.add, replica_groups=PAIRS,
                                 ins=[gin[:].opt()], outs=[gout[:].opt()])
    gf = tp.tile([128, 3], F32, tag="gf", bufs=2, name="gf")
    nc.sync.dma_start(out=gf[:], in_=gout[:])
    gbf3 = tp.tile([128, 3], BF16, tag="gbf3", bufs=2, name="gbf3")
    nc.vector.tensor_copy(gbf3[:], gf[:])

    # ---- tn1 x-part matmuls for the first 3 chunks, emitted BEFORE the
    # collective-dependent geo matmuls so the in-order PE queue overlaps
    # them with the all-reduce latency ----
    t1 = [tp.tile([128, SH], BF16, tag=f"t1_{ni}", bufs=1, name=f"t1_{ni}") for ni in range(5)]
    t1ps = []
    for ni, (n0, n1) in list(enumerate(chunks(640)))[:3]:
        ps = ps_work(pp, "ps_t1")
        nc.tensor.matmul(ps[: n1 - n0, 0:SH], tw1_t[0][:128, n0:n1], xs1[0][:128, :], start=True, stop=False)
        nc.tensor.matmul(ps[: n1 - n0, 0:SH], tw1_t[1][:128, n0:n1], xs1[1][:128, :], start=False, stop=False)
        nc.tensor.matmul(ps[: n1 - n0, 0:SH], tw1_t[2][:64, n0:n1], xs1[2][:64, :], start=False, stop=True)
        t1ps.append(ps)

    # tp column [17,1]: wcat^T @ g + bcat
    gps_a = pp.tile([128, SH], F32, tag="ps_av0", bufs=1, name="gps_a")
    gps_b = pp.tile([128, SH], F32, tag="ps_av1", bufs=1, name="gps_b")
    warm_keep(nc, ones, gps_b[32:33, 0:SH], 56)
    for ci, (c0, c1) in enumerate(DCH):
        nc.tensor.matmul(gps_a[0:17, 0:1], wcat_t[ci][: c1 - c0, :], gbf3[: c1 - c0, ci:ci + 1],
                         start=(ci == 0), stop=False)
    nc.tensor.matmul(gps_a[0:17, 0:1], bcat_t[:], ones[0:1, 0:1], start=False, stop=True)
    # softmax groups [0:4),[4:12),[14:17); tanh [12:14) — all on the column
    exc = tp.tile([128, 1], BF16, tag="exc", bufs=2, name="exc")
    nc.scalar.activation(exc[0:17, :], gps_a[0:17, 0:1], AF.Exp)
    nc.tensor.matmul(gps_b[0:3, 0:1], msk[:, 0:3], exc[0:17, :], start=True, stop=True)
    rg = tp.tile([3, 1], F32, tag="rg", bufs=2, name="rg")
    nc.vector.reciprocal(rg[:], gps_b[0:3, 0:1])
    rgb = tp.tile([3, 1], BF16, tag="rgb", bufs=2, name="rgb")
    nc.vector.tensor_copy(rgb[:], rg[:])
    nc.tensor.matmul(gps_b[0:17, 4:5], mskT[:, :], rgb[:], start=True, stop=True)
    # softmax part (rows 12:14 scale to 0) + tanh part merged via the mask column
    tps = tp.tile([128, 1], BF16, tag="tps", bufs=2, name="tps")
    nc.vector.tensor_tensor(out=tps[0:17, :], in0=exc[0:17, :], in1=gps_b[0:17, 4:5], op=OP.mult)
    tha = tp.tile([128, 1], BF16, tag="tha", bufs=2, name="tha")
    nc.scalar.activation(tha[0:17, :], gps_a[0:17, 0:1], AF.Tanh)
    tpc = tp.tile([128, 1], BF16, tag="tpc", bufs=2, name="tpc")
    nc.vector.scalar_tensor_tensor(out=tpc[0:17, :], in0=tha[0:17, :], scalar=msk[0:17, 3:4],
                                   in1=tps[0:17, :], op0=OP.mult, op1=OP.add)
    # cvec[640] = tw1_tp^T @ tp  (5 chunk columns) + tb1 -> effective t1 bias
    for ni, (n0, n1) in enumerate(chunks(640)):
        nc.tensor.matmul(gps_a[: n1 - n0, 16 + ni:17 + ni], tw1t_t[0:17, n0:n1], tpc[0:17, :],
                         start=True, stop=True)
    t1b = tp.tile([128, 5], F32, tag="t1b", bufs=2, name="t1b")
    nc.vector.tensor_tensor(out=t1b[:], in0=gps_a[:, 16:21], in1=tb1_w, op=OP.add)

    # ---- tn1 evictions + remaining chunks ----
    for ni, (n0, n1) in list(enumerate(chunks(640)))[:3]:
        nc.scalar.activation(t1[ni][: n1 - n0, :], t1ps[ni][: n1 - n0, 0:SH], AF.Gelu,
                             bias=t1b[: n1 - n0, ni:ni + 1], scale=1.0)
    for ni, (n0, n1) in list(enumerate(chunks(640)))[3:]:
        ps = ps_work(pp, "ps_t1")
        nc.tensor.matmul(ps[: n1 - n0, 0:SH], tw1_t[0][:128, n0:n1], xs1[0][:128, :], start=True, stop=False)
        nc.tensor.matmul(ps[: n1 - n0, 0:SH], tw1_t[1][:128, n0:n1], xs1[1][:128, :], start=False, stop=False)
        nc.tensor.matmul(ps[: n1 - n0, 0:SH], tw1_t[2][:64, n0:n1], xs1[2][:64, :], start=False, stop=True)
        nc.scalar.activation(t1[ni][: n1 - n0, :], ps[: n1 - n0, 0:SH], AF.Gelu,
                             bias=t1b[: n1 - n0, ni:ni + 1], scale=1.0)
    t2 = [tp.tile([128, SH], BF16, tag=f"t2_{ni}", bufs=1, name=f"t2_{ni}") for ni in range(3)]

    def evict_t2(ps, ni, n0, n1):
        nc.scalar.activation(t2[ni][: n1 - n0, :], ps[: n1 - n0, 0:SH], AF.Gelu,
                             bias=tb2_c[ni], scale=1.0)

    mm_proj(nc, pp, tw2_t, t1, 640, D, evict_t2)
    xs2 = [ap_.tile([128, SH], BF16, tag=f"xs2_{ci}", bufs=1, name=f"xs2_{ci}") for ci in range(3)]

    def evict_t3(ps, ni, n0, n1):
        nc.vector.scalar_tensor_tensor(out=xs2[ni][: n1 - n0, :], in0=ps[: n1 - n0, 0:SH],
                                       scalar=tb3_c[ni], in1=xs1[ni][: n1 - n0, :],
                                       op0=OP.add, op1=OP.add)

    mm_proj(nc, pp, tw3_t, t2, D, D, evict_t3)

    # ---- ff MLP + post-LN ----
    f1 = [tp.tile([128, SH], BF16, tag=f"f1_{ni}", bufs=1, name=f"f1_{ni}") for ni in range(10)]

    def evict_f1(ps, ni, n0, n1):
        nc.scalar.activation(f1[ni][: n1 - n0, :], ps[: n1 - n0, 0:SH], AF.Gelu,
                             bias=fb1_c[ni], scale=1.0)

    mm_proj(nc, pp, fw1_t, xs2, D, FFD, evict_f1)
    res2 = [tp.tile([128, SH], BF16, tag=f"res{ci}", bufs=2, name=f"res2_{ci}") for ci in range(3)]

    def evict_f2(ps, ni, n0, n1):
        nc.vector.scalar_tensor_tensor(out=res2[ni][: n1 - n0, :], in0=ps[: n1 - n0, 0:SH],
                                       scalar=fb2_c[ni], in1=xs2[ni][: n1 - n0, :],
                                       op0=OP.add, op1=OP.add)

    mm_proj(nc, pp, fw2_t, f1, FFD, D, evict_f2)
    return layernorm(nc, ap_, tp, pp, res2, l2g_c, l2b_c, ones, ones_f, eps_t, "xsb",
                     ones_row=True)


def layernorm(nc, ap_, tp, pp, res, g_c, b_c, ones, ones_f, eps_t, tag, ones_row=False):
    """LN over the feature (partition) dim of res (3 chunk tiles [kc, SH] bf16).
    Per-token stats are computed on [1, SH] rows, then broadcast to [128, SH]
    via two f32 matmuls for the partition-parallel normalize passes."""
    ps_s = ps_work(pp, "ps_s")
    ps_q = ps_work(pp, "ps_q")
    for ci, (c0, c1) in enumerate(DCH):
        kc = c1 - c0
        sq = tp.tile([128, SH], BF16, tag=f"sq{ci}", bufs=1, name=f"sq{ci}")
        nc.scalar.square(sq[:kc, :], res[ci][:kc, :])
        nc.tensor.matmul(ps_s[:1, 0:SH], ones[:kc, 0:1], res[ci][:kc, :],
                         start=(ci == 0), stop=(ci == 2))
        nc.tensor.matmul(ps_q[:1, 0:SH], ones[:kc, 0:1], sq[:kc, :],
                         start=(ci == 0), stop=(ci == 2))
    mrow = tp.tile([1, SH], F32, tag="m_row", bufs=1, name="mrow")
    nc.vector.tensor_scalar(out=mrow[:], in0=ps_s[:1, 0:SH], scalar1=1.0 / D, scalar2=None,
                            op0=OP.mult)
    m2 = tp.tile([1, SH], F32, tag="m2_row", bufs=1, name="m2")
    nc.vector.tensor_tensor(out=m2[:], in0=mrow[:], in1=mrow[:], op=OP.mult)
    varr = tp.tile([1, SH], F32, tag="var_row", bufs=1, name="varr")
    nc.vector.scalar_tensor_tensor(out=varr[:], in0=ps_q[:1, 0:SH], scalar=1.0 / D,
                                   in1=m2[:], op0=OP.mult, op1=OP.subtract)
    sd = tp.tile([1, SH], F32, tag="sd_row", bufs=1, name="sd")
    nc.scalar.activation(sd[:], varr[:], AF.Sqrt, bias=eps_t[0:1, 0:1])
    rstd = tp.tile([1, SH], F32, tag="rstd_row", bufs=1, name="rstd")
    nc.vector.reciprocal_approx_fast(rstd[:], sd[:])
    ps_bm = pp.tile([128, SH], F32, tag="ps_av0", bufs=1, name="ps_bm")
    nc.tensor.matmul(ps_bm[:], ones_f[0:1, :], mrow[:], start=True, stop=True)
    ps_br = pp.tile([128, SH], F32, tag="ps_av1", bufs=1, name="ps_br")
    nc.tensor.matmul(ps_br[:], ones_f[0:1, :], rstd[:], start=True, stop=True)
    out = [ap_.tile([128, SH], BF16, tag=f"{tag}{ci}", bufs=1, name=f"{tag}{ci}") for ci in range(3)]
    for ci, (c0, c1) in enumerate(DCH):
        kc = c1 - c0
        tmp = tp.tile([128, SH], BF16, tag="lnt", bufs=1, name="lnt")
        nc.vector.tensor_tensor(out=tmp[:kc, :], in0=res[ci][:kc, :], in1=ps_bm[:kc, :],
                                op=OP.subtract)
        nc.vector.scalar_tensor_tensor(out=out[ci][:kc, :], in0=tmp[:kc, :], scalar=g_c[ci],
                                       in1=ps_br[:kc, :], op0=OP.mult, op1=OP.mult)
        nc.vector.tensor_scalar(out=out[ci][:kc, :], in0=out[ci][:kc, :], scalar1=b_c[ci],
                                scalar2=None, op0=OP.add)
    if ones_row:
        nc.vector.memset(out[2][64:65, :], 1.0)
    return out


def conv_head(nc, cp, wp, ap_, tp, pp, dram, P, xs, vec_aps, ones, ones_f):
    cb_t = [vec_aps(f"cb{kk_i}", D) for kk_i in range(4)]
    cwg = {}

    def get_cw(tap_):
        g = tap_ // 2
        if g not in cwg:
            cwg.clear()
            cwg[g] = load_w(nc, wp, P["ck"], g, D, 2 * D, "cw", bufs=3)
        return cwg[g], D * (tap_ % 2)

    feats = []

    # 1x1 conv straight from the resident activations — no gather dependency;
    # runs while the pair AllGather + pad construction are in flight
    cw0, coff0 = get_cw(0)
    ps_c1 = [None] * 3
    for ni, (n0, n1) in enumerate(DCH):
        ps_c1[ni] = ps_work(pp, "ps_c1")
        for ci, (c0, c1) in enumerate(DCH):
            nc.tensor.matmul(ps_c1[ni][: n1 - n0, 0:SH], cw0[ci][: c1 - c0, coff0 + n0:coff0 + n1],
                             xs[ci][: c1 - c0, :], start=(ci == 0), stop=(ci == 2))
    for ni, (n0, n1) in enumerate(DCH):
        ft = ap_.tile([128, SH], BF16, tag=f"ft0_{ni}", bufs=1, name=f"ft0_{ni}")
        nc.scalar.activation(ft[: n1 - n0, :], ps_c1[ni][: n1 - n0, 0:SH], AF.Relu,
                             bias=cb_t[0][ni], scale=1.0)
        feats.append(ft)
    ps_wk = ps_work(pp, "ps_wk")
    warm_keep(nc, ones, ps_wk[0:1, 0:SH], 72)

    # gather final xs across the pair
    xin = dram.tile([D, SH], BF16, tag="xin", bufs=1, name="xin")
    xout = dram.tile([2, D, SH], BF16, tag="xout", bufs=1, name="xout")
    for ci, (c0, c1) in enumerate(DCH):
        nc.sync.dma_start(out=xin[c0:c1, :], in_=xs[ci][: c1 - c0, :])
    nc.gpsimd.collective_compute("AllGather", OP.bypass, replica_groups=PAIRS,
                                 ins=[xin[:].opt()], outs=[xout[:].opt()])
    # padded full grid + own 21x36 window, all in SBUF (one dynamic-offset DVE copy)
    off_e = {}
    for eng_ in (nc.vector, nc.gpsimd):
        off_e[eng_] = (eng_.partition_id() % 2) * (15 * PG)
    pad = [ap_.tile([128, 21 * PG], BF16, tag=f"pad{ci}", bufs=1, name=f"pad{ci}") for ci in range(3)]
    for ci, (c0, c1) in enumerate(DCH):
        kc = c1 - c0
        eng = nc.gpsimd if ci == 1 else nc.vector
        xfull = ap_.tile([128, S], BF16, tag="xfull", bufs=1, name="xfull")
        nc.sync.dma_start(out=xfull[:kc].rearrange("p (g q) -> p g q", g=2),
                          in_=xout[:, c0:c1, :].rearrange("g p q -> p g q"))
        xpadf = ap_.tile([128, PG * PG], BF16, tag=f"xpadf{ci % 2}", bufs=1, name="xpadf")
        eng.memset(xpadf[:kc], 0.0)
        eng.tensor_copy(xpadf[:kc].rearrange("p (r c) -> p r c", r=PG)[:, 3:3 + G, 3:3 + G],
                        xfull[:kc].rearrange("p (r c) -> p r c", r=G))
        eng.tensor_copy(pad[ci][:kc, :], xpadf[:kc, bass.ds(off_e[eng], 21 * PG)])

    tap = 1
    cv_tags = ["ps_av0", "ps_av1"]
    for kk_i, kk in enumerate((3, 5, 7)):
        r = kk // 2
        ntaps = kk * kk
        ps_cv = [pp.tile([128, SH], F32, tag=cv_tags[0], bufs=1, name="ps_cv0"),
                 pp.tile([128, SH], F32, tag=cv_tags[1], bufs=1, name="ps_cv1"),
                 ps_work(pp, "ps_cv2")]
        for ti in range(ntaps):
            dy, dx = ti // kk - r, ti % kk - r
            cw, coff = get_cw(tap)
            tap += 1
            # contiguous shifted copies: strided-rhs matmuls never reach the
            # 2.4 GHz p-state; DVE assembles [128, SH] tiles the PE can stream
            rsh = []
            for ci, (c0, c1) in enumerate(DCH):
                t_ = tp.tile([128, SH], BF16, tag=f"rsh{ci}", bufs=3, name=f"rsh{ci}")
                eng = nc.gpsimd if ci == 2 else nc.vector
                eng.tensor_copy(
                    t_[: c1 - c0, :].rearrange("p (r c) -> p r c", r=15),
                    pad[ci][: c1 - c0, :].rearrange("p (r c) -> p r c", r=21)[
                        :, 3 + dy:18 + dy, 3 + dx:3 + dx + G])
                rsh.append(t_)
            for ni, (n0, n1) in enumerate(DCH):
                out_ps = ps_cv[ni] if ni < 2 else ps_cv[2]
                for ci, (c0, c1) in enumerate(DCH):
                    nc.tensor.matmul(out_ps[: n1 - n0, 0:SH], cw[ci][: c1 - c0, coff + n0:coff + n1],
                                     rsh[ci][: c1 - c0, :], start=(ti == 0 and ci == 0),
                                     stop=(ti == ntaps - 1 and ci == 2))
        for ni, (n0, n1) in enumerate(DCH):
            out_ps = ps_cv[ni] if ni < 2 else ps_cv[2]
            ft = ap_.tile([128, SH], BF16, tag=f"ft{kk_i + 1}_{ni}", bufs=1, name=f"ft{kk_i + 1}_{ni}")
            nc.scalar.activation(ft[: n1 - n0, :], out_ps[: n1 - n0, 0:SH], AF.Relu,
                                 bias=cb_t[kk_i + 1][ni], scale=1.0)
            feats.append(ft)

    # fus: [1280 -> 320], contraction chunks follow the feat tile boundaries
    fch = []
    row = 0
    for kk_i in range(4):
        for ci, (c0, c1) in enumerate(DCH):
            fch.append((row, row + (c1 - c0)))
            row += c1 - c0
    fus_t = load_w(nc, wp, P["fusw"], None, FFD, D, "fusw", bufs=1, boundaries=fch)
    fusb_c = vec_aps("fusb", D)
    fused = [tp.tile([128, SH], BF16, tag=f"fused{ni}", bufs=1, name=f"fused{ni}") for ni in range(3)]
    for ni, (n0, n1) in enumerate(DCH):
        ps = ps_work(pp, "ps_fus")
        for ci, (r0, r1) in enumerate(fch):
            nc.tensor.matmul(ps[: n1 - n0, 0:SH], fus_t[ci][: r1 - r0, n0:n1], feats[ci][: r1 - r0, :],
                             start=(ci == 0), stop=(ci == len(fch) - 1))
        nc.vector.tensor_scalar(out=fused[ni][: n1 - n0, :], in0=ps[: n1 - n0, 0:SH],
                                scalar1=fusb_c[ni], scalar2=None, op0=OP.add)

    # output head
    def head_mm(X, wname, bname, kdim, ndim, gelu, name, out_dt=BF16):
        wt = load_w(nc, wp, P[wname], None, kdim, ndim, name, bufs=1)
        bt = vec_aps(bname, ndim)
        outs = [tp.tile([128, SH], out_dt, tag=f"{name}o{ni}", bufs=1, name=f"{name}o{ni}")
                for ni in range(len(chunks(ndim)))]

        def ev(ps, ni, n0, n1):
            if gelu:
                nc.scalar.activation(outs[ni][: n1 - n0, :], ps[: n1 - n0, 0:SH],
                                     AF.Gelu, bias=bt[ni], scale=1.0)
            else:
                nc.vector.tensor_scalar(out=outs[ni][: n1 - n0, :], in0=ps[: n1 - n0, 0:SH],
                                        scalar1=bt[ni], scalar2=None, op0=OP.add)

        mm_proj(nc, pp, wt, X, kdim, ndim, ev)
        return outs

    h1 = head_mm(fused, "ow1", "ob1", D, 160, True, "ow1")
    h2 = head_mm(h1, "ow2", "ob2", 160, 80, True, "ow2")
    lg = head_mm(h2, "ow3", "ob3", 80, 10, False, "ow3", out_dt=F32)  # [10, SH] f32

    nc.sync.dma_start(out=P["out"][:], in_=lg[0][:10, :])


# ======================= host side =======================

def prep_inputs(inputs):
    """Full inputs -> list of 8 per-core input dicts."""
    import ml_dtypes
    bf16 = ml_dtypes.bfloat16
    f32 = np.float32
    ip = {k: np.asarray(v) for k, v in inputs.items()}

    def bf(x):
        return np.ascontiguousarray(np.asarray(x, f32)).astype(bf16)

    com = {}
    com["iota10"] = np.arange(10, dtype=f32).reshape(10, 1)
    com["inw"] = bf(ip["in_emb_w"])

    # head-padded q/k/v/o layouts (64-wide slot per head; v has the sum slot at 64h)
    wqp = np.zeros((L, D, HP), f32)
    wkp = np.zeros((L, D, HP), f32)
    wvp = np.zeros((L, D, HP), f32)
    wop = np.zeros((L, HP, D), f32)
    bqp = np.zeros((L, HP), f32)
    bkp = np.zeros((L, HP), f32)
    bvp = np.zeros((L, HP), f32)
    for h in range(NH):
        hs = slice(40 * h, 40 * h + 40)
        wqp[:, :, 64 * h:64 * h + 40] = ip["wq"][:, :, hs]
        wkp[:, :, 64 * h:64 * h + 40] = ip["wk"][:, :, hs]
        wvp[:, :, 64 * h + 1:64 * h + 41] = ip["wv"][:, :, hs]
        wop[:, 64 * h + 1:64 * h + 41, :] = ip["wo"][:, hs, :]
        bqp[:, 64 * h:64 * h + 40] = ip["bq"][:, hs] * SCALE
        bkp[:, 64 * h:64 * h + 40] = ip["bk"][:, hs]
        bvp[:, 64 * h] = 1.0
        bvp[:, 64 * h + 1:64 * h + 41] = ip["bv"][:, hs]
    wsrc = {"wq": wqp, "wk": wkp, "wv": wvp, "wo": wop}
    wsrc["wcat"] = np.concatenate([ip["w_rot"], ip["w_refl"], ip["w_tr"], ip["w_sc"]],
                                  axis=2) * (1.0 / S)
    wsrc["tw3"] = ip["tn_w3"] * 0.3
    wsrc["fw1"] = ip["ff_w1"]
    com["bcat"] = bf(np.concatenate([ip["b_rot"], ip["b_refl"], ip["b_tr"], ip["b_sc"]],
                                    axis=1).reshape(L, 1, 17))
    mk = np.zeros((17, 4), f32)
    mk[0:4, 0] = 1.0
    mk[4:12, 1] = 1.0
    mk[14:17, 2] = 1.0
    mk[12:14, 3] = 1.0
    com["mskp"] = bf(mk)
    mkT = np.zeros((3, 17), f32)
    mkT[0, 0:4] = 1.0
    mkT[1, 4:12] = 1.0
    mkT[2, 14:17] = 1.0
    com["mskTp"] = bf(mkT)
    wsrc["tw1"] = ip["tn_w1"][:, :D, :]
    wsrc["tw1t"] = np.ascontiguousarray(ip["tn_w1"][:, D:D + 17, :])
    wsrc["tw2"] = ip["tn_w2"]
    wsrc["fw2"] = ip["ff_w2"]

    def pack_w(specs, cm, ncols):
        pk = np.zeros((L, 128, ncols), f32)
        for name, kdim, ndim, bnd in specs:
            arr = np.asarray(wsrc[name], f32)
            for ci, (c0, c1) in enumerate(bnd or chunks(kdim)):
                off = cm[(name, ci)]
                pk[:, : c1 - c0, off:off + ndim] = arr[:, c0:c1, :]
        return pk

    pkA = pack_w(WPK_A, WPKA_CM, WPKA_COLS)
    # v bias + softmax-sum one-slots ride contraction row 64 of the wv chunk-2 tile
    offv = WPKA_CM[("wv", 2)]
    pkA[:, 64, offv:offv + HP] = bvp
    com["wpackA"] = bf(pkA)
    com["wpackM"] = bf(pack_w(WPK_M, WPKM_CM, WPKM_COLS))
    taps = np.concatenate([ip["ck1"].reshape(1, D, D), ip["ck3"].reshape(9, D, D),
                           ip["ck5"].reshape(25, D, D), ip["ck7"].reshape(49, D, D)], axis=0)
    com["ck"] = bf(taps.reshape(NTAPG, 2, D, D).transpose(0, 2, 1, 3).reshape(NTAPG, D, 2 * D))
    com["fusw"] = bf(ip["fus_w"])
    com["ow1"], com["ow2"], com["ow3"] = bf(ip["op_w1"]), bf(ip["op_w2"]), bf(ip["op_w3"])

    # packed per-partition vectors
    vec_src = {}
    for l in range(L):
        vec_src[("bqs", l)] = bqp[l]
        vec_src[("bk", l)] = bkp[l]
        vec_src[("bo", l)] = ip["bo"][l]
        vec_src[("lag", l)] = ip["ln_a_g"][l]
        vec_src[("lab", l)] = ip["ln_a_b"][l]
        vec_src[("l2g", l)] = ip["ln2_g"][l]
        vec_src[("l2b", l)] = ip["ln2_b"][l]
        vec_src[("tb1", l)] = ip["tn_b1"][l]
        vec_src[("tb2", l)] = ip["tn_b2"][l]
        vec_src[("tb3", l)] = ip["tn_b3"][l] * 0.3
        vec_src[("fb1", l)] = ip["ff_b1"][l]
        vec_src[("fb2", l)] = ip["ff_b2"][l]
    vec_src[("inb", None)] = ip["in_emb_b"]
    for i, kk in enumerate((1, 3, 5, 7)):
        vec_src[(f"cb{i}", None)] = ip[f"cb{kk}"]
    vec_src[("fusb", None)] = ip["fus_b"]
    vec_src[("ob1", None)] = ip["op_b1"]
    vec_src[("ob2", None)] = ip["op_b2"]
    vec_src[("ob3", None)] = ip["op_b3"]
    vecpack = np.zeros((128, VEC_NCOL), f32)
    for (name, l, ci), col in VEC_COLMAP.items():
        src = np.asarray(vec_src[(name, l)], f32)
        c0, c1 = chunks(len(src))[ci]
        vecpack[: c1 - c0, col] = src[c0:c1]
    com["vecpack"] = vecpack

    # geometric bias, transposed + kchunk-padded + head-pair interleaved:
    # [l, kchunk, g2, r, 2*q]; per core, key rows are reordered [own | peer]
    dist_idx, dir_idx = ip["dist_idx"], ip["dir_idx"]
    bhkq_l = []
    for l in range(L):
        bqk = ip["dist_emb"][l][dist_idx] + ip["dir_emb"][l][dir_idx]   # [q, k, h] f32
        bhkq_l.append(np.ascontiguousarray(bqk.transpose(2, 1, 0)) * BIAS_SCALE)  # [h, k, q]
    bias_half = []
    f8 = None
    import ml_dtypes as _md
    f8 = _md.float8_e4m3fn
    for half in range(2):
        own = slice(SH * half, SH * half + SH)
        peer_s = slice(SH * (1 - half), SH * (1 - half) + SH)
        bt = np.zeros((L, len(KCH), 128, NH // 2, 2 * SH), dtype=f8)
        for l in range(L):
            ordered = np.concatenate([bhkq_l[l][:, own, own], bhkq_l[l][:, peer_s, own]], axis=1)
            for ci, (k0, k1) in enumerate(KCH):
                for g2 in range(NH // 2):
                    bt[l, ci, : k1 - k0, g2, 0:SH] = ordered[2 * g2, k0:k1, :].astype(f8)
                    bt[l, ci, : k1 - k0, g2, SH:2 * SH] = ordered[2 * g2 + 1, k0:k1, :].astype(f8)
        bias_half.append(bt)

    peT_full = np.ascontiguousarray(ip["pe"].reshape(S, D).T.astype(f32))  # [D, S]
    grids = ip["input_grid"].reshape(B, S)

    in_maps = []
    for c in range(8):
        b, half = c // 2, c % 2
        t0, t1 = SH * half, SH * (1 - half)
        m = dict(com)
        m["grid"] = np.concatenate([grids[b, t0:t0 + SH], grids[b, t1:t1 + SH]]
                                   ).astype(f32).reshape(1, S).astype(bf16)
        m["peT"] = bf(np.concatenate([peT_full[:, t0:t0 + SH], peT_full[:, t1:t1 + SH]], axis=1))
        m["biasT"] = bias_half[half]
        in_maps.append(m)
    return in_maps


_BUILT = None


def _fuse_ldweights(nc):
    """Drop tile_legalize's explicit InstLdweights (the paired InstMatmult is
    still self-loading); keep their sync waits/updates on EventSemaphores so
    walrus can compile with --enable-ldw-opt=true and background the loads."""
    for f in nc.m.functions:
        for bb in f.blocks:
            il = bb.instructions
            newlist = []
            changed = False
            for i, ins in enumerate(il):
                if type(ins).__name__ == "InstLdweights":
                    changed = True
                    if i + 1 < len(il) and type(il[i + 1]).__name__ == "InstMatmult":
                        il[i + 1].ldweights = True   # matmul self-loads now
                    si = ins.sync_info
                    nw = len(si.on_wait) if si else 0
                    nu = len(si.on_update) if si else 0
                    if nw == 0 and nu == 0:
                        continue
                    ev = mybir.InstEventSemaphore(
                        name=f"ldwev_{ins.name}", engine=ins.engine,
                        ins=[], outs=[], sync_info=si, debug=ins.debug)
                    newlist.append(ev)
                    continue
                newlist.append(ins)
            if changed:
                bb.instructions = newlist


def get_built():
    global _BUILT
    if _BUILT is None:
        import os
        nc = bacc.Bacc("TRN2", target_bir_lowering=False, num_devices=8)
        build(nc)
        nc.finalize()
        if os.environ.get("ATLAS_LDWFUSE") == "1":
            _fuse_ldweights(nc)
        _BUILT = nc
    return _BUILT


_LDW_PATCHED = False


def _enable_ldw_opt():
    """Compile NEFFs with --enable-ldw-opt=true (overlaps LDWEIGHTS with matmuls)."""
    global _LDW_PATCHED
    if _LDW_PATCHED:
        return
    import concourse.bass_utils as bu
    orig = bu.run_command

    def patched(cmd, cwd=None, **kw):
        cmd = ["--enable-ldw-opt=true" if c == "--enable-ldw-opt=false" else c for c in cmd]
        return orig(cmd, cwd=cwd, **kw)

    bu.run_command = patched
    _LDW_PATCHED = True


def kernel(**inputs):
    from concourse.bass_utils import run_bass_kernel_spmd
    import os
    if os.environ.get("ATLAS_LDWOPT") == "1":
        _enable_ldw_opt()
    nc = get_built()
    in_maps = prep_inputs(inputs)
    trace = bool(os.environ.get("ATLAS_TRACE"))
    res = run_bass_kernel_spmd(nc, in_maps, core_ids=list(range(8)), trace=trace)
    if trace:
        kernel.last_exec_time_ns = res.exec_time_ns
        kernel.last_results = res
    out = np.zeros((B, G, G, 10), np.float32)
    for c in range(8):
        b, half = c // 2, c % 2
        out[b, 15 * half:15 * half + 15] = res.results[c]["out"].T.reshape(15, G, 10)
    return out


# revision 49
# speedup vs baseline: 1.0096x; 1.0096x over previous
"""AtlasV4Transformer Trainium2 kernel — 8-core SPMD, token-split data parallel.

Sharding: core c -> batch b = c//2, token half = c%2 (450 of 900 grid tokens).
Activations are feature-major on chip: x^T [D(partitions, 3 chunk tiles), tokens].
Attention uses transposed scores S^T[k,q] per head; softmax row sums ride along
the AV matmul via a constant-1 slot built into the head-padded V layout (head h
occupies a 64-wide slot: [1 | v(40) | 0]).  Head pairs are processed jointly:
the two score matmuls of a pair row-pack the PE array (contraction rows 0:40 /
64:104) into one 2-bank PSUM tile, bias-add runs as one DVE pass and exp as one
ACT pass over [kc, 900].  The geometric-transform AllReduce feeds a rank-17
bias column (cvec) folded into tn1's gelu bias, so the tn MLP matmuls never
wait on the collective.  The per-head geometric bias table gather is
materialized on the host and streamed as a pre-interleaved fp8 input.
"""
import sys

import numpy as np

if "/opt/trn_rl_repo" not in sys.path:
    sys.path.insert(0, "/opt/trn_rl_repo")

import concourse.bass as bass
import concourse.bacc as bacc
import concourse.mybir as mybir
from concourse import tile

F32 = mybir.dt.float32
BF16 = mybir.dt.bfloat16
F8 = mybir.dt.float8e4
BIAS_SCALE = 64.0
AF = mybir.ActivationFunctionType
OP = mybir.AluOpType

B, G, D, L, NH, DK, S = 4, 30, 320, 4, 8, 40, 900
SH = S // 2            # tokens owned per core
FFD = 4 * D            # 1280
HP = 512               # head-padded q/k/v width (8 heads x 64)
SCALE = 1.0 / np.sqrt(DK)
EPS = 1e-5
PAIRS = [[0, 1], [2, 3], [4, 5], [6, 7]]
PG = G + 6             # padded grid 36
NTAPG = 42             # conv taps packed 2 per group (84 total)


def chunks(n, c=128):
    return [(i, min(i + c, n)) for i in range(0, n, c)]


DCH = chunks(D)          # 3 feature chunks
# key-token chunks aligned to the 450/450 own|peer split
KCH = chunks(SH) + [(SH + a, SH + b) for (a, b) in chunks(SH)]

# packed per-layer weight blocks: fixed column order shared by host and device
TB1 = [(0, 128), (128, 256), (256, 320)]
WPK_A = [("wq", D, HP, None), ("wk", D, HP, None), ("wv", D, HP, None), ("wo", HP, D, None)]
WPK_M = [("wcat", D, 17, None), ("tw1", D, 640, TB1), ("tw1t", 17, 640, [(0, 17)]),
         ("tw2", 640, D, None), ("tw3", D, D, None), ("fw1", D, FFD, None), ("fw2", FFD, D, None)]


def build_wpk_colmap(specs):
    cm, col = {}, 0
    for name, kdim, ndim, bnd in specs:
        for ci, _ in enumerate(bnd or chunks(kdim)):
            cm[(name, ci)] = col
            col += ndim
    return cm, col


WPKA_CM, WPKA_COLS = build_wpk_colmap(WPK_A)
WPKM_CM, WPKM_COLS = build_wpk_colmap(WPK_M)


def wpk_slices(t, specs, cm):
    out = {}
    for name, kdim, ndim, bnd in specs:
        out[name] = [t[:, cm[(name, ci)]:cm[(name, ci)] + ndim]
                     for ci, _ in enumerate(bnd or chunks(kdim))]
    return out


# packed per-partition vectors: fixed column order shared by host and device
VEC_LAYER_SPECS = [("bqs", HP), ("bk", HP), ("bo", D), ("lag", D), ("lab", D),
                   ("l2g", D), ("l2b", D), ("tb1", 640), ("tb2", D), ("tb3", D),
                   ("fb1", FFD), ("fb2", D)]
VEC_GLOBAL_SPECS = [("inb", D), ("cb0", D), ("cb1", D), ("cb2", D), ("cb3", D),
                    ("fusb", D), ("ob1", 160), ("ob2", 80), ("ob3", 10)]


def build_vec_colmap():
    cm = {}
    col = 0
    for l in range(L):
        for name, n in VEC_LAYER_SPECS:
            for ci in range(len(chunks(n))):
                cm[(name, l, ci)] = col
                col += 1
    for name, n in VEC_GLOBAL_SPECS:
        for ci in range(len(chunks(n))):
            cm[(name, None, ci)] = col
            col += 1
    return cm, col


VEC_COLMAP, VEC_NCOL = build_vec_colmap()


def build(nc):
    dpi = lambda name, shape, dt: nc.declare_dram_parameter(name, list(shape), dt, isOutput=False)

    P = {}
    P["grid"] = dpi("grid", [1, S], BF16)
    P["iota10"] = dpi("iota10", [10, 1], F32)
    P["peT"] = dpi("peT", [D, S], BF16)
    P["inw"] = dpi("inw", [10, D], BF16)
    P["vecpack"] = dpi("vecpack", [128, VEC_NCOL], F32)
    P["wpackA"] = dpi("wpackA", [L, 128, WPKA_COLS], BF16)
    P["wpackM"] = dpi("wpackM", [L, 128, WPKM_COLS], BF16)
    P["bcat"] = dpi("bcat", [L, 1, 17], BF16)
    P["mskp"] = dpi("mskp", [17, 4], BF16)
    P["mskTp"] = dpi("mskTp", [3, 17], BF16)
    # bias, transposed + chunk-padded + host-interleaved: [l, kchunk, r(128), g2, 2*q(450)]
    # (g2 innermost so one contiguous DMA per (kchunk, gp) covers both heads of both j's)
    P["biasT"] = dpi("biasT", [L, len(KCH), 128, NH // 2, 2 * SH], F8)
    P["ck"] = dpi("ck", [NTAPG, D, 2 * D], BF16)
    P["fusw"] = dpi("fusw", [FFD, D], BF16)
    P["ow1"] = dpi("ow1", [D, 160], BF16)
    P["ow2"] = dpi("ow2", [160, 80], BF16)
    P["ow3"] = dpi("ow3", [80, 10], BF16)
    P["out"] = nc.declare_dram_parameter("out", [10, SH], F32, isOutput=True)

    with tile.TileContext(nc) as tc:
        with (
            tc.tile_pool(name="const", bufs=1) as cp,
            tc.tile_pool(name="wts", bufs=1) as wp,
            tc.tile_pool(name="acts", bufs=1) as ap_,
            tc.tile_pool(name="tmp", bufs=1) as tp,
            tc.tile_pool(name="psum", bufs=1, space="PSUM") as pp,
            tc.tile_pool(name="dram", bufs=1, space="DRAM") as dram,
        ):
            build_body(nc, tc, cp, wp, ap_, tp, pp, dram, P)
    return nc


def load_w(nc, wp, param, l, kdim, ndim, name, bufs=1, boundaries=None):
    ts = []
    for ci, (c0, c1) in enumerate(boundaries or chunks(kdim)):
        t = wp.tile([128, ndim], BF16, tag=f"{name}{ci}", bufs=bufs, name=f"{name}{ci}")
        src = param[l, c0:c1, :] if l is not None else param[c0:c1, :]
        nc.sync.dma_start(out=t[: c1 - c0, :], in_=src)
        ts.append(t)
    return ts


def ps_work(pp, name="ps_w"):
    """One 2-bank [128, 1024] f32 PSUM tile from the shared rotation."""
    return pp.tile([128, 1024], F32, tag="ps_w", bufs=3, name=name)


def warm_keep(nc, ones, trash, n):
    """Dependency-free filler matmuls into a throwaway PSUM row.  Emitted into
    known PE-idle windows (collective wait, conv pad construction) so the HAM
    activity monitor keeps the array at the 2.4 GHz clock state."""
    for _ in range(n):
        nc.tensor.matmul(trash, ones[0:128, 0:1], ones[:, 0:SH], start=True, stop=True)


def mm_proj(nc, pp, Wt, X, kdim, ndim, evict):
    kch = chunks(kdim)
    for ni, (n0, n1) in enumerate(chunks(ndim)):
        ps = ps_work(pp, "ps_mm")
        for ci, (c0, c1) in enumerate(kch):
            nc.tensor.matmul(ps[: n1 - n0, 0:SH], Wt[ci][: c1 - c0, n0:n1], X[ci][: c1 - c0, :],
                             start=(ci == 0), stop=(ci == len(kch) - 1))
        evict(ps, ni, n0, n1)


def build_body(nc, tc, cp, wp, ap_, tp, pp, dram, P):
    # ---------------- constants ----------------
    ones = cp.tile([128, SH], BF16, tag="ones", bufs=1, name="ones")
    nc.vector.memset(ones[:], 1.0)
    ones_f = cp.tile([1, 128], F32, tag="ones_f", bufs=1, name="ones_f")
    nc.vector.memset(ones_f[:], 1.0)
    # softmax group masks for the 17-wide geometric transform column (host consts)
    msk = cp.tile([17, 4], BF16, tag="msk", bufs=1, name="msk")
    nc.sync.dma_start(out=msk[:], in_=P["mskp"][:])
    mskT = cp.tile([3, 17], BF16, tag="mskT", bufs=1, name="mskT")
    nc.sync.dma_start(out=mskT[:], in_=P["mskTp"][:])

    eps_t = cp.tile([1, 1], F32, tag="eps", bufs=1, name="eps_t")
    nc.vector.memset(eps_t[:], EPS)
    iota_t = cp.tile([10, 1], F32, tag="iota", bufs=1, name="iota_t")
    nc.sync.dma_start(out=iota_t[:], in_=P["iota10"][:])
    peT_t = [cp.tile([128, S], BF16, tag=f"peT{ci}", bufs=1, name=f"peT{ci}") for ci in range(3)]
    for ci, (c0, c1) in enumerate(DCH):
        nc.sync.dma_start(out=peT_t[ci][: c1 - c0, :], in_=P["peT"][c0:c1, :])
    grid_t = cp.tile([1, S], BF16, tag="grid", bufs=1, name="grid_t")
    nc.sync.dma_start(out=grid_t[:], in_=P["grid"][:])
    inw_t = cp.tile([10, D], BF16, tag="inw", bufs=1, name="inw_t")
    nc.sync.dma_start(out=inw_t[:], in_=P["inw"][:])
    vp = cp.tile([128, VEC_NCOL], F32, tag="vecpack", bufs=1, name="vp")
    nc.sync.dma_start(out=vp[:], in_=P["vecpack"][:])

    def vec_aps(name, n, l=None):
        return [vp[: c1 - c0, VEC_COLMAP[(name, l, ci)]:VEC_COLMAP[(name, l, ci)] + 1]
                for ci, (c0, c1) in enumerate(chunks(n))]

    def vec_wide(name, l, n):
        col0 = VEC_COLMAP[(name, l, 0)]
        return vp[:, col0:col0 + n]

    bcat_t = []
    for l in range(L):
        t2_ = cp.tile([1, 17], BF16, tag=f"bcat{l}", bufs=1, name=f"bcat{l}")
        nc.sync.dma_start(out=t2_[:], in_=P["bcat"][l])
        bcat_t.append(t2_)

    # ---------------- embedding (both halves; kills the layer-0 gather) ----------------
    oh = tp.tile([10, S], BF16, tag="oh", bufs=1, name="oh")
    for half in range(2):
        hs = slice(SH * half, SH * half + SH)
        ps_g = ps_work(pp, "ps_g")
        nc.tensor.matmul(ps_g[:10, 0:SH], ones[0:1, 0:10], grid_t[0:1, hs], start=True, stop=True)
        nc.vector.tensor_scalar(out=oh[:, hs], in0=ps_g[:10, 0:SH], scalar1=iota_t[:10, :],
                                scalar2=None, op0=OP.is_equal)

    inb_c = vec_aps("inb", D)
    xs = [ap_.tile([128, SH], BF16, tag=f"xs{ci}", bufs=1, name=f"xs{ci}") for ci in range(3)]
    xp0 = [ap_.tile([128, SH], BF16, tag=f"xp{ci}", bufs=1, name=f"xp{ci}") for ci in range(3)]
    for ci, (c0, c1) in enumerate(DCH):
        for half in range(2):
            hs = slice(SH * half, SH * half + SH)
            dst = xs[ci] if half == 0 else xp0[ci]
            pse = ps_work(pp, "pse")
            nc.tensor.matmul(pse[: c1 - c0, 0:SH], inw_t[:, c0:c1], oh[:, hs], start=True, stop=True)
            nc.vector.scalar_tensor_tensor(
                out=dst[: c1 - c0, :], in0=pse[: c1 - c0, 0:SH], scalar=inb_c[ci],
                in1=peT_t[ci][: c1 - c0, hs], op0=OP.add, op1=OP.add)
    # ones row for the v-bias contraction trick
    nc.vector.memset(xs[2][64:65, :], 1.0)
    nc.vector.memset(xp0[2][64:65, :], 1.0)

    # ---------------- transformer layers ----------------
    W = load_attn_w(nc, wp, P, 0)
    for l in range(L):
        Wnext = {} if l + 1 < L else None
        xs = layer(nc, wp, ap_, tp, pp, dram, P, l, xs, vec_aps, vec_wide, bcat_t[l],
                   ones, ones_f, msk, mskT, eps_t, xp0 if l == 0 else None, W, Wnext)
        W = Wnext

    # ---------------- conv fusion + head ----------------
    conv_head(nc, cp, wp, ap_, tp, pp, dram, P, xs, vec_aps, ones, ones_f)


def load_attn_w(nc, wp, P, l):
    """q/k/v/o weights as ONE packed DMA, prefetched one layer ahead (bufs=2)."""
    t = wp.tile([128, WPKA_COLS], BF16, tag="wpkA", bufs=2, name="wpkA")
    nc.sync.dma_start(out=t[:], in_=P["wpackA"][l])
    return wpk_slices(t, WPK_A, WPKA_CM)


def layer(nc, wp, ap_, tp, pp, dram, P, l, xs, vec_aps, vec_wide, bcat_t, ones, ones_f,
          msk, mskT, eps_t, xp0, W, Wnext):
    bqs_c = vec_aps("bqs", HP, l)
    bk_c = vec_aps("bk", HP, l)
    bo_c = vec_aps("bo", D, l)
    lag_c = vec_aps("lag", D, l)
    lab_c = vec_aps("lab", D, l)
    l2g_c = vec_aps("l2g", D, l)
    l2b_c = vec_aps("l2b", D, l)
    tb2_c = vec_aps("tb2", D, l)
    tb3_c = vec_aps("tb3", D, l)
    fb1_c = vec_aps("fb1", FFD, l)
    fb2_c = vec_aps("fb2", D, l)
    tb1_w = vec_wide("tb1", l, 5)

    wq_t, wk_t, wv_t, wo_t = W["wq"], W["wk"], W["wv"], W["wo"]
    # MLP weights: one packed DMA issued at layer top, lands under attention
    wm = wp.tile([128, WPKM_COLS], BF16, tag="wpkM", bufs=1, name="wpkM")
    nc.sync.dma_start(out=wm[:], in_=P["wpackM"][l])
    WM = wpk_slices(wm, WPK_M, WPKM_CM)
    wcat_t, tw1_t, tw1t_t, tw2_t = WM["wcat"], WM["tw1"], WM["tw1t"][0], WM["tw2"]
    tw3_t, fw1_t, fw2_t = WM["tw3"], WM["fw1"], WM["fw2"]
    if Wnext is not None:
        Wnext.update(load_attn_w(nc, wp, P, l + 1))

    # ---- q projection, scaled; head-padded rows [64h, 64h+40) ----
    qp = [ap_.tile([128, SH], BF16, tag=f"qp{g}", bufs=1, name=f"qp{g}") for g in range(4)]

    def evict_q(ps, ni, n0, n1):
        nc.vector.tensor_scalar(out=qp[ni][: n1 - n0, :], in0=ps[: n1 - n0, 0:SH],
                                scalar1=SCALE, scalar2=bqs_c[ni], op0=OP.mult, op1=OP.add)

    mm_proj(nc, pp, wq_t, xs, D, HP, evict_q)

    # ---- peer-x gather (single collective; layer 0 has xp precomputed) ----
    if xp0 is not None:
        xp = xp0
    else:
        xgin = dram.tile([D, SH], BF16, tag="xgin", bufs=2, name="xgin")
        xgout = dram.tile([2, D, SH], BF16, tag="xgout", bufs=2, name="xgout")
        for ci, (c0, c1) in enumerate(DCH):
            nc.sync.dma_start(out=xgin[c0:c1, :], in_=xs[ci][: c1 - c0, :])
        nc.gpsimd.collective_compute("AllGather", OP.bypass, replica_groups=PAIRS,
                                     ins=[xgin[:].opt()], outs=[xgout[:].opt()])
        peer = (nc.sync.partition_id() + 1) % 2
        xgout_f = xgout[:].rearrange("g p q -> (g p) q")
        xp = [ap_.tile([128, SH], BF16, tag=f"xp{ci}", bufs=1, name=f"xp{ci}") for ci in range(3)]
        for ci, (c0, c1) in enumerate(DCH):
            nc.sync.dma_start(out=xp[ci][: c1 - c0, :],
                              in_=xgout_f[bass.ds(peer * D + c0, c1 - c0), :])
        nc.vector.memset(xp[2][64:65, :], 1.0)

    # ---- k for all 900 keys, feature-major [own cols | peer cols] ----
    khp = [ap_.tile([128, S], BF16, tag=f"khp{g2}", bufs=1, name=f"khp{g2}") for g2 in range(4)]

    def evict_k_own(ps, ni, n0, n1):
        nc.vector.tensor_scalar(out=khp[ni][: n1 - n0, 0:SH], in0=ps[: n1 - n0, 0:SH],
                                scalar1=bk_c[ni], scalar2=None, op0=OP.add)

    mm_proj(nc, pp, wk_t, xs, D, HP, evict_k_own)

    def evict_k_peer(ps, ni, n0, n1):
        nc.vector.tensor_scalar(out=khp[ni][: n1 - n0, SH:S], in0=ps[: n1 - n0, 0:SH],
                                scalar1=bk_c[ni], scalar2=None, op0=OP.add)

    # ---- v, token-major rows [own | peer], 64-wide head slots; the v bias and
    # the softmax-sum 1-slots ride contraction row 64 of chunk 2 (ones row) ----
    def v_chunks(rng):
        for si in rng:
            k0, k1 = KCH[si]
            kc = k1 - k0
            src_x, off = (xs, 0) if k1 <= SH else (xp, SH)
            psv = ps_work(pp, "psv")
            for ci, (c0, c1) in enumerate(DCH):
                kk = (c1 - c0) + (1 if ci == 2 else 0)
                nc.tensor.matmul(psv[:kc, 0:HP], src_x[ci][:kk, k0 - off:k1 - off],
                                 wv_t[ci][:kk, :], start=(ci == 0), stop=(ci == 2))
            t = ap_.tile([128, HP], BF16, tag=f"va{si}", bufs=1, name=f"va{si}")
            nc.vector.tensor_copy(t[:kc, :], psv[:kc, 0:HP])
            va.append(t)

    va = []
    v_chunks(range(4))
    mm_proj(nc, pp, wk_t, xp, D, HP, evict_k_peer)
    v_chunks(range(4, 8))

    # ---- attention: head pairs processed jointly; all own-key chunks (ci<4)
    # run before any peer-key dependency, hiding the x-gather latency ----
    attnT = [ap_.tile([128, SH], BF16, tag=f"at{g}", bufs=2, name=f"at{g}") for g in range(4)]
    for gp in range(2):
        ps_avs = [pp.tile([128, SH], F32, tag=f"ps_av{j}", bufs=1, name=f"ps_av{j}")
                  for j in range(2)]

        def emit_av(j, ci, kc, ee):
            # col-packed AV pair: the two heads use disjoint 32-col groups
            a0 = 128 * (2 * gp + j)
            nc.tensor.matmul(ps_avs[j][0:64, :], va[ci][:kc, a0:a0 + 64],
                             ee[:kc, 0:SH], start=(ci == 0), stop=(ci == len(KCH) - 1),
                             tile_position=(0, 0))
            nc.tensor.matmul(ps_avs[j][64:128, :], va[ci][:kc, a0 + 64:a0 + 128],
                             ee[:kc, SH:2 * SH], start=(ci == 0), stop=(ci == len(KCH) - 1),
                             tile_position=(0, 64))

        pend = []   # AV pairs lag their scores by 1 unit
        for ci, (k0, k1) in enumerate(KCH):
            kc = k1 - k0
            btp = tp.tile([128, 4 * SH], F8, tag="bias", bufs=2, name="btp")
            nc.gpsimd.dma_start(
                out=btp[:kc, :],
                in_=P["biasT"][l, ci, :kc, 2 * gp:2 * gp + 2, :].rearrange("p g q -> p (g q)"))
            for j in range(2):
                g2 = 2 * gp + j
                bt2 = btp[:, 2 * SH * j:2 * SH * j + 2 * SH]
                psp = ps_work(pp, "psp")
                # row-packed score pair: contraction rows 0:40 and 64:104 run
                # concurrently in disjoint 32-row groups of the PE array
                nc.tensor.matmul(psp[:kc, 0:SH], khp[g2][0:40, k0:k1], qp[g2][0:40, :],
                                 start=True, stop=True, tile_position=(0, 0))
                nc.tensor.matmul(psp[:kc, 512:512 + SH], khp[g2][64:104, k0:k1],
                                 qp[g2][64:104, :], start=True, stop=True,
                                 tile_position=(64, 0))
                es = tp.tile([128, 2 * SH], BF16, tag="esc", bufs=2, name="es")
                nc.vector.scalar_tensor_tensor(
                    out=es[:kc].rearrange("p (h q) -> p h q", h=2),
                    in0=bt2[:kc].rearrange("p (h q) -> p h q", h=2),
                    scalar=1.0 / BIAS_SCALE,
                    in1=psp[:kc].rearrange("p (h q) -> p h q", h=2)[:, :, 0:SH],
                    op0=OP.mult, op1=OP.add)
                ee = tp.tile([128, 2 * SH], BF16, tag="eexp", bufs=2, name="ee")
                nc.scalar.activation(ee[:kc, :], es[:kc, :], AF.Exp)
                pend.append((j, ci, kc, ee))
                if len(pend) > 1:
                    emit_av(*pend.pop(0))
        for u in pend:
            emit_av(*u)
        for j in range(2):
            g2 = 2 * gp + j
            ps_av = ps_avs[j]
            # sum rows 0 / 64 -> SBUF rows, broadcast to partition halves via
            # matmul, then reciprocal runs partition-parallel on [128, SH]
            s2a = tp.tile([1, SH], BF16, tag="rec", bufs=2, name="s2a")
            s2b = tp.tile([1, SH], BF16, tag="recb", bufs=2, name="s2b")
            nc.vector.tensor_copy(s2a[:], ps_av[0:1, :])
            nc.vector.tensor_copy(s2b[:], ps_av[64:65, :])
            ps_bc = ps_work(pp, "ps_bc")
            nc.tensor.matmul(ps_bc[0:64, 0:SH], ones[0:1, 0:64], s2a[:], start=True, stop=True)
            nc.tensor.matmul(ps_bc[64:128, 0:SH], ones[0:1, 0:64], s2b[:], start=True, stop=True)
            bc = tp.tile([128, SH], F32, tag="bcn", bufs=1, name="bc")
            nc.vector.reciprocal_approx_fast(bc[:], ps_bc[:, 0:SH])
            nc.vector.tensor_tensor(out=attnT[g2][:], in0=ps_av[:], in1=bc[:], op=OP.mult)

    # ---- wo projection + residual + LN ----
    res = [tp.tile([128, SH], BF16, tag=f"res{ci}", bufs=2, name=f"res{ci}") for ci in range(3)]

    def evict_o(ps, ni, n0, n1):
        nc.vector.scalar_tensor_tensor(out=res[ni][: n1 - n0, :], in0=ps[: n1 - n0, 0:SH],
                                       scalar=bo_c[ni], in1=xs[ni][: n1 - n0, :],
                                       op0=OP.add, op1=OP.add)

    mm_proj(nc, pp, wo_t, attnT, HP, D, evict_o)
    xs1 = layernorm(nc, ap_, tp, pp, res, lag_c, lab_c, ones, ones_f, eps_t, "xsa")

    # ---- geometric transform: pair all-reduce -> 17-wide transform params ->
    # rank-17 contribution becomes a per-partition bias column for tn1 ----
    gin = dram.tile([128, 3], F32, tag="gin", bufs=2, name="gin")
    gout = dram.tile([128, 3], F32, tag="gout", bufs=2, name="gout")
    gred = tp.tile([128, 3], F32, tag="gred", bufs=2, name="gred")
    for ci, (c0, c1) in enumerate(DCH):
        nc.vector.reduce_sum(gred[: c1 - c0, ci:ci + 1], xs1[ci][: c1 - c0, :],
                             axis=mybir.AxisListType.X)
    nc.sync.dma_start(out=gin[:], in_=gred[:])
    nc.gpsimd.collective_compute("AllReduce", OP.add, replica_groups=PAIRS,
                                 ins=[gin[:].opt()], outs=[gout[:].opt()])
    gf = tp.tile([128, 3], F32, tag="gf", bufs=2, name="gf")
    nc.sync.dma_start(out=gf[:], in_=gout[:])
    gbf3 = tp.tile([128, 3], BF16, tag="gbf3", bufs=2, name="gbf3")
    nc.vector.tensor_copy(gbf3[:], gf[:])

    # ---- tn1 x-part matmuls for the first 3 chunks, emitted BEFORE the
    # collective-dependent geo matmuls so the in-order PE queue overlaps
    # them with the all-reduce latency ----
    t1 = [tp.tile([128, SH], BF16, tag=f"t1_{ni}", bufs=1, name=f"t1_{ni}") for ni in range(5)]
    t1ps = []
    for ni, (n0, n1) in list(enumerate(chunks(640)))[:3]:
        ps = ps_work(pp, "ps_t1")
        nc.tensor.matmul(ps[: n1 - n0, 0:SH], tw1_t[0][:128, n0:n1], xs1[0][:128, :], start=True, stop=False)
        nc.tensor.matmul(ps[: n1 - n0, 0:SH], tw1_t[1][:128, n0:n1], xs1[1][:128, :], start=False, stop=False)
        nc.tensor.matmul(ps[: n1 - n0, 0:SH], tw1_t[2][:64, n0:n1], xs1[2][:64, :], start=False, stop=True)
        t1ps.append(ps)

    # tp column [17,1]: wcat^T @ g + bcat
    gps_a = pp.tile([128, SH], F32, tag="ps_av0", bufs=1, name="gps_a")
    gps_b = pp.tile([128, SH], F32, tag="ps_av1", bufs=1, name="gps_b")
    warm_keep(nc, ones, gps_b[32:33, 0:SH], 56)
    for ci, (c0, c1) in enumerate(DCH):
        nc.tensor.matmul(gps_a[0:17, 0:1], wcat_t[ci][: c1 - c0, :], gbf3[: c1 - c0, ci:ci + 1],
                         start=(ci == 0), stop=False)
    nc.tensor.matmul(gps_a[0:17, 0:1], bcat_t[:], ones[0:1, 0:1], start=False, stop=True)
    # softmax groups [0:4),[4:12),[14:17); tanh [12:14) — all on the column
    exc = tp.tile([128, 1], BF16, tag="exc", bufs=2, name="exc")
    nc.scalar.activation(exc[0:17, :], gps_a[0:17, 0:1], AF.Exp)
    nc.tensor.matmul(gps_b[0:3, 0:1], msk[:, 0:3], exc[0:17, :], start=True, stop=True)
    rg = tp.tile([3, 1], F32, tag="rg", bufs=2, name="rg")
    nc.vector.reciprocal(rg[:], gps_b[0:3, 0:1])
    rgb = tp.tile([3, 1], BF16, tag="rgb", bufs=2, name="rgb")
    nc.vector.tensor_copy(rgb[:], rg[:])
    nc.tensor.matmul(gps_b[0:17, 4:5], mskT[:, :], rgb[:], start=True, stop=True)
    # softmax part (rows 12:14 scale to 0) + tanh part merged via the mask column
    tps = tp.tile([128, 1], BF16, tag="tps", bufs=2, name="tps")
    nc.vector.tensor_tensor(out=tps[0:17, :], in0=exc[0:17, :], in1=gps_b[0:17, 4:5], op=OP.mult)
    tha = tp.tile([128, 1], BF16, tag="tha", bufs=2, name="tha")
    nc.scalar.activation(tha[0:17, :], gps_a[0:17, 0:1], AF.Tanh)
    tpc = tp.tile([128, 1], BF16, tag="tpc", bufs=2, name="tpc")
    nc.vector.scalar_tensor_tensor(out=tpc[0:17, :], in0=tha[0:17, :], scalar=msk[0:17, 3:4],
                                   in1=tps[0:17, :], op0=OP.mult, op1=OP.add)
    # cvec[640] = tw1_tp^T @ tp  (5 chunk columns) + tb1 -> effective t1 bias
    for ni, (n0, n1) in enumerate(chunks(640)):
        nc.tensor.matmul(gps_a[: n1 - n0, 16 + ni:17 + ni], tw1t_t[0:17, n0:n1], tpc[0:17, :],
                         start=True, stop=True)
    t1b = tp.tile([128, 5], F32, tag="t1b", bufs=2, name="t1b")
    nc.vector.tensor_tensor(out=t1b[:], in0=gps_a[:, 16:21], in1=tb1_w, op=OP.add)

    # ---- tn1 evictions + remaining chunks ----
    for ni, (n0, n1) in list(enumerate(chunks(640)))[:3]:
        nc.scalar.activation(t1[ni][: n1 - n0, :], t1ps[ni][: n1 - n0, 0:SH], AF.Gelu,
                             bias=t1b[: n1 - n0, ni:ni + 1], scale=1.0)
    for ni, (n0, n1) in list(enumerate(chunks(640)))[3:]:
        ps = ps_work(pp, "ps_t1")
        nc.tensor.matmul(ps[: n1 - n0, 0:SH], tw1_t[0][:128, n0:n1], xs1[0][:128, :], start=True, stop=False)
        nc.tensor.matmul(ps[: n1 - n0, 0:SH], tw1_t[1][:128, n0:n1], xs1[1][:128, :], start=False, stop=False)
        nc.tensor.matmul(ps[: n1 - n0, 0:SH], tw1_t[2][:64, n0:n1], xs1[2][:64, :], start=False, stop=True)
        nc.scalar.activation(t1[ni][: n1 - n0, :], ps[: n1 - n0, 0:SH], AF.Gelu,
                             bias=t1b[: n1 - n0, ni:ni + 1], scale=1.0)
    t2 = [tp.tile([128, SH], BF16, tag=f"t2_{ni}", bufs=1, name=f"t2_{ni}") for ni in range(3)]

    def evict_t2(ps, ni, n0, n1):
        nc.scalar.activation(t2[ni][: n1 - n0, :], ps[: n1 - n0, 0:SH], AF.Gelu,
                             bias=tb2_c[ni], scale=1.0)

    mm_proj(nc, pp, tw2_t, t1, 640, D, evict_t2)
    xs2 = [ap_.tile([128, SH], BF16, tag=f"xs2_{ci}", bufs=1, name=f"xs2_{ci}") for ci in range(3)]

    def evict_t3(ps, ni, n0, n1):
        nc.vector.scalar_tensor_tensor(out=xs2[ni][: n1 - n0, :], in0=ps[: n1 - n0, 0:SH],
                                       scalar=tb3_c[ni], in1=xs1[ni][: n1 - n0, :],
                                       op0=OP.add, op1=OP.add)

    mm_proj(nc, pp, tw3_t, t2, D, D, evict_t3)

    # ---- ff MLP + post-LN ----
    f1 = [tp.tile([128, SH], BF16, tag=f"f1_{ni}", bufs=1, name=f"f1_{ni}") for ni in range(10)]

    def evict_f1(ps, ni, n0, n1):
        nc.scalar.activation(f1[ni][: n1 - n0, :], ps[: n1 - n0, 0:SH], AF.Gelu,
                             bias=fb1_c[ni], scale=1.0)

    mm_proj(nc, pp, fw1_t, xs2, D, FFD, evict_f1)
    res2 = [tp.tile([128, SH], BF16, tag=f"res{ci}", bufs=2, name=f"res2_{ci}") for ci in range(3)]

    def evict_f2(ps, ni, n0, n1):
        nc.vector.scalar_tensor_tensor(out=res2[ni][: n1 - n0, :], in0=ps[: n1 - n0, 0:SH],
                                       scalar=fb2_c[ni], in1=xs2[ni][: n1 - n0, :],
                                       op0=OP.add, op1=OP.add)

    mm_proj(nc, pp, fw2_t, f1, FFD, D, evict_f2)
    return layernorm(nc, ap_, tp, pp, res2, l2g_c, l2b_c, ones, ones_f, eps_t, "xsb",
                     ones_row=True)


def layernorm(nc, ap_, tp, pp, res, g_c, b_c, ones, ones_f, eps_t, tag, ones_row=False):
    """LN over the feature (partition) dim of res (3 chunk tiles [kc, SH] bf16).
    Per-token stats are computed on [1, SH] rows, then broadcast to [128, SH]
    via two f32 matmuls for the partition-parallel normalize passes."""
    ps_s = ps_work(pp, "ps_s")
    ps_q = ps_work(pp, "ps_q")
    for ci, (c0, c1) in enumerate(DCH):
        kc = c1 - c0
        sq = tp.tile([128, SH], BF16, tag=f"sq{ci}", bufs=1, name=f"sq{ci}")
        nc.scalar.square(sq[:kc, :], res[ci][:kc, :])
        nc.tensor.matmul(ps_s[:1, 0:SH], ones[:kc, 0:1], res[ci][:kc, :],
                         start=(ci == 0), stop=(ci == 2))
        nc.tensor.matmul(ps_q[:1, 0:SH], ones[:kc, 0:1], sq[:kc, :],
                         start=(ci == 0), stop=(ci == 2))
    mrow = tp.tile([1, SH], F32, tag="m_row", bufs=1, name="mrow")
    nc.vector.tensor_scalar(out=mrow[:], in0=ps_s[:1, 0:SH], scalar1=1.0 / D, scalar2=None,
                            op0=OP.mult)
    m2 = tp.tile([1, SH], F32, tag="m2_row", bufs=1, name="m2")
    nc.vector.tensor_tensor(out=m2[:], in0=mrow[:], in1=mrow[:], op=OP.mult)
    varr = tp.tile([1, SH], F32, tag="var_row", bufs=1, name="varr")
    nc.vector.scalar_tensor_tensor(out=varr[:], in0=ps_q[:1, 0:SH], scalar=1.0 / D,
                                   in1=m2[:], op0=OP.mult, op1=OP.subtract)
    sd = tp.tile([1, SH], F32, tag="sd_row", bufs=1, name="sd")
    nc.scalar.activation(sd[:], varr[:], AF.Sqrt, bias=eps_t[0:1, 0:1])
    rstd = tp.tile([1, SH], F32, tag="rstd_row", bufs=1, name="rstd")
    nc.vector.reciprocal_approx_fast(rstd[:], sd[:])
    ps_bm = pp.tile([128, SH], F32, tag="ps_av0", bufs=1, name="ps_bm")
    nc.tensor.matmul(ps_bm[:], ones_f[0:1, :], mrow[:], start=True, stop=True)
    ps_br = pp.tile([128, SH], F32, tag="ps_av1", bufs=1, name="ps_br")
    nc.tensor.matmul(ps_br[:], ones_f[0:1, :], rstd[:], start=True, stop=True)
    out = [ap_.tile([128, SH], BF16, tag=f"{tag}{ci}", bufs=1, name=f"{tag}{ci}") for ci in range(3)]
    for ci, (c0, c1) in enumerate(DCH):
        kc = c1 - c0
        tmp = tp.tile([128, SH], BF16, tag="lnt", bufs=1, name="lnt")
        nc.vector.tensor_tensor(out=tmp[:kc, :], in0=res[ci][:kc, :], in1=ps_bm[:kc, :],
                                op=OP.subtract)
        nc.vector.scalar_tensor_tensor(out=out[ci][:kc, :], in0=tmp[:kc, :], scalar=g_c[ci],
                                       in1=ps_br[:kc, :], op0=OP.mult, op1=OP.mult)
        nc.vector.tensor_scalar(out=out[ci][:kc, :], in0=out[ci][:kc, :], scalar1=b_c[ci],
                                scalar2=None, op0=OP.add)
    if ones_row:
        nc.vector.memset(out[2][64:65, :], 1.0)
    return out


def conv_head(nc, cp, wp, ap_, tp, pp, dram, P, xs, vec_aps, ones, ones_f):
    cb_t = [vec_aps(f"cb{kk_i}", D) for kk_i in range(4)]
    # prefetch the fusion + head weights now; they land during the conv blocks
    fch = []
    row = 0
    for kk_i in range(4):
        for ci, (c0, c1) in enumerate(DCH):
            fch.append((row, row + (c1 - c0)))
            row += c1 - c0
    fus_t = load_w(nc, wp, P["fusw"], None, FFD, D, "fusw", bufs=1, boundaries=fch)
    ow_t = {nm: load_w(nc, wp, P[nm], None, kd, nd, nm, bufs=1)
            for nm, kd, nd in (("ow1", D, 160), ("ow2", 160, 80), ("ow3", 80, 10))}
    cwg = {}

    def get_cw(tap_):
        g = tap_ // 2
        if g not in cwg:
            cwg.clear()
            cwg[g] = load_w(nc, wp, P["ck"], g, D, 2 * D, "cw", bufs=3)
        return cwg[g], D * (tap_ % 2)

    feats = []

    # 1x1 conv straight from the resident activations — no gather dependency;
    # runs while the pair AllGather + pad construction are in flight
    cw0, coff0 = get_cw(0)
    ps_c1 = [None] * 3
    for ni, (n0, n1) in enumerate(DCH):
        ps_c1[ni] = ps_work(pp, "ps_c1")
        for ci, (c0, c1) in enumerate(DCH):
            nc.tensor.matmul(ps_c1[ni][: n1 - n0, 0:SH], cw0[ci][: c1 - c0, coff0 + n0:coff0 + n1],
                             xs[ci][: c1 - c0, :], start=(ci == 0), stop=(ci == 2))
    for ni, (n0, n1) in enumerate(DCH):
        ft = ap_.tile([128, SH], BF16, tag=f"ft0_{ni}", bufs=1, name=f"ft0_{ni}")
        nc.scalar.activation(ft[: n1 - n0, :], ps_c1[ni][: n1 - n0, 0:SH], AF.Relu,
                             bias=cb_t[0][ni], scale=1.0)
        feats.append(ft)
    ps_wk = ps_work(pp, "ps_wk")
    warm_keep(nc, ones, ps_wk[0:1, 0:SH], 72)

    # gather final xs across the pair
    xin = dram.tile([D, SH], BF16, tag="xin", bufs=1, name="xin")
    xout = dram.tile([2, D, SH], BF16, tag="xout", bufs=1, name="xout")
    for ci, (c0, c1) in enumerate(DCH):
        nc.sync.dma_start(out=xin[c0:c1, :], in_=xs[ci][: c1 - c0, :])
    nc.gpsimd.collective_compute("AllGather", OP.bypass, replica_groups=PAIRS,
                                 ins=[xin[:].opt()], outs=[xout[:].opt()])
    # padded full grid + own 21x36 window, all in SBUF (one dynamic-offset DVE copy)
    off_e = {}
    for eng_ in (nc.vector, nc.gpsimd):
        off_e[eng_] = (eng_.partition_id() % 2) * (15 * PG)
    pad = [ap_.tile([128, 21 * PG], BF16, tag=f"pad{ci}", bufs=1, name=f"pad{ci}") for ci in range(3)]
    for ci, (c0, c1) in enumerate(DCH):
        kc = c1 - c0
        eng = nc.gpsimd if ci == 1 else nc.vector
        xfull = ap_.tile([128, S], BF16, tag="xfull", bufs=1, name="xfull")
        nc.sync.dma_start(out=xfull[:kc].rearrange("p (g q) -> p g q", g=2),
                          in_=xout[:, c0:c1, :].rearrange("g p q -> p g q"))
        xpadf = ap_.tile([128, PG * PG], BF16, tag=f"xpadf{ci % 2}", bufs=1, name="xpadf")
        eng.memset(xpadf[:kc], 0.0)
        eng.tensor_copy(xpadf[:kc].rearrange("p (r c) -> p r c", r=PG)[:, 3:3 + G, 3:3 + G],
                        xfull[:kc].rearrange("p (r c) -> p r c", r=G))
        eng.tensor_copy(pad[ci][:kc, :], xpadf[:kc, bass.ds(off_e[eng], 21 * PG)])

    tap = 1
    cv_tags = ["ps_av0", "ps_av1"]
    for kk_i, kk in enumerate((3, 5, 7)):
        r = kk // 2
        ntaps = kk * kk
        ps_cv = [pp.tile([128, SH], F32, tag=cv_tags[0], bufs=1, name="ps_cv0"),
                 pp.tile([128, SH], F32, tag=cv_tags[1], bufs=1, name="ps_cv1"),
                 ps_work(pp, "ps_cv2")]
        # re-pin the clock state across the block boundary (accumulator swap +
        # first shifted-copy latency is where warmth has been observed to die)
        warm_keep(nc, ones, ps_cv[2][64:65, 0:SH], 12)
        for ti in range(ntaps):
            dy, dx = ti // kk - r, ti % kk - r
            cw, coff = get_cw(tap)
            tap += 1
            # contiguous shifted copies: strided-rhs matmuls never reach the
            # 2.4 GHz p-state; DVE assembles [128, SH] tiles the PE can stream
            rsh = []
            for ci, (c0, c1) in enumerate(DCH):
                t_ = tp.tile([128, SH], BF16, tag=f"rsh{ci}", bufs=3, name=f"rsh{ci}")
                eng = nc.gpsimd if ci == 2 else nc.vector
                eng.tensor_copy(
                    t_[: c1 - c0, :].rearrange("p (r c) -> p r c", r=15),
                    pad[ci][: c1 - c0, :].rearrange("p (r c) -> p r c", r=21)[
                        :, 3 + dy:18 + dy, 3 + dx:3 + dx + G])
                rsh.append(t_)
            for ni, (n0, n1) in enumerate(DCH):
                out_ps = ps_cv[ni] if ni < 2 else ps_cv[2]
                for ci, (c0, c1) in enumerate(DCH):
                    nc.tensor.matmul(out_ps[: n1 - n0, 0:SH], cw[ci][: c1 - c0, coff + n0:coff + n1],
                                     rsh[ci][: c1 - c0, :], start=(ti == 0 and ci == 0),
                                     stop=(ti == ntaps - 1 and ci == 2))
        for ni, (n0, n1) in enumerate(DCH):
            out_ps = ps_cv[ni] if ni < 2 else ps_cv[2]
            ft = ap_.tile([128, SH], BF16, tag=f"ft{kk_i + 1}_{ni}", bufs=1, name=f"ft{kk_i + 1}_{ni}")
            nc.scalar.activation(ft[: n1 - n0, :], out_ps[: n1 - n0, 0:SH], AF.Relu,
                                 bias=cb_t[kk_i + 1][ni], scale=1.0)
            feats.append(ft)

    # fus: [1280 -> 320], contraction chunks follow the feat tile boundaries
    fusb_c = vec_aps("fusb", D)
    fused = [tp.tile([128, SH], BF16, tag=f"fused{ni}", bufs=1, name=f"fused{ni}") for ni in range(3)]
    for ni, (n0, n1) in enumerate(DCH):
        ps = ps_work(pp, "ps_fus")
        for ci, (r0, r1) in enumerate(fch):
            nc.tensor.matmul(ps[: n1 - n0, 0:SH], fus_t[ci][: r1 - r0, n0:n1], feats[ci][: r1 - r0, :],
                             start=(ci == 0), stop=(ci == len(fch) - 1))
        nc.vector.tensor_scalar(out=fused[ni][: n1 - n0, :], in0=ps[: n1 - n0, 0:SH],
                                scalar1=fusb_c[ni], scalar2=None, op0=OP.add)

    # output head
    def head_mm(X, wname, bname, kdim, ndim, gelu, name, out_dt=BF16):
        wt = ow_t[wname]
        bt = vec_aps(bname, ndim)
        outs = [tp.tile([128, SH], out_dt, tag=f"{name}o{ni}", bufs=1, name=f"{name}o{ni}")
                for ni in range(len(chunks(ndim)))]

        def ev(ps, ni, n0, n1):
            if gelu:
                nc.scalar.activation(outs[ni][: n1 - n0, :], ps[: n1 - n0, 0:SH],
                                     AF.Gelu, bias=bt[ni], scale=1.0)
            else:
                nc.vector.tensor_scalar(out=outs[ni][: n1 - n0, :], in0=ps[: n1 - n0, 0:SH],
                                        scalar1=bt[ni], scalar2=None, op0=OP.add)

        mm_proj(nc, pp, wt, X, kdim, ndim, ev)
        return outs

    h1 = head_mm(fused, "ow1", "ob1", D, 160, True, "ow1")
    h2 = head_mm(h1, "ow2", "ob2", 160, 80, True, "ow2")
    lg = head_mm(h2, "ow3", "ob3", 80, 10, False, "ow3", out_dt=F32)  # [10, SH] f32

    nc.sync.dma_start(out=P["out"][:], in_=lg[0][:10, :])


# ======================= host side =======================

def prep_inputs(inputs):
    """Full inputs -> list of 8 per-core input dicts."""
    import ml_dtypes
    bf16 = ml_dtypes.bfloat16
    f32 = np.float32
    ip = {k: np.asarray(v) for k, v in inputs.items()}

    def bf(x):
        return np.ascontiguousarray(np.asarray(x, f32)).astype(bf16)

    com = {}
    com["iota10"] = np.arange(10, dtype=f32).reshape(10, 1)
    com["inw"] = bf(ip["in_emb_w"])

    # head-padded q/k/v/o layouts (64-wide slot per head; v has the sum slot at 64h)
    wqp = np.zeros((L, D, HP), f32)
    wkp = np.zeros((L, D, HP), f32)
    wvp = np.zeros((L, D, HP), f32)
    wop = np.zeros((L, HP, D), f32)
    bqp = np.zeros((L, HP), f32)
    bkp = np.zeros((L, HP), f32)
    bvp = np.zeros((L, HP), f32)
    for h in range(NH):
        hs = slice(40 * h, 40 * h + 40)
        wqp[:, :, 64 * h:64 * h + 40] = ip["wq"][:, :, hs]
        wkp[:, :, 64 * h:64 * h + 40] = ip["wk"][:, :, hs]
        wvp[:, :, 64 * h + 1:64 * h + 41] = ip["wv"][:, :, hs]
        wop[:, 64 * h + 1:64 * h + 41, :] = ip["wo"][:, hs, :]
        bqp[:, 64 * h:64 * h + 40] = ip["bq"][:, hs] * SCALE
        bkp[:, 64 * h:64 * h + 40] = ip["bk"][:, hs]
        bvp[:, 64 * h] = 1.0
        bvp[:, 64 * h + 1:64 * h + 41] = ip["bv"][:, hs]
    wsrc = {"wq": wqp, "wk": wkp, "wv": wvp, "wo": wop}
    wsrc["wcat"] = np.concatenate([ip["w_rot"], ip["w_refl"], ip["w_tr"], ip["w_sc"]],
                                  axis=2) * (1.0 / S)
    wsrc["tw3"] = ip["tn_w3"] * 0.3
    wsrc["fw1"] = ip["ff_w1"]
    com["bcat"] = bf(np.concatenate([ip["b_rot"], ip["b_refl"], ip["b_tr"], ip["b_sc"]],
                                    axis=1).reshape(L, 1, 17))
    mk = np.zeros((17, 4), f32)
    mk[0:4, 0] = 1.0
    mk[4:12, 1] = 1.0
    mk[14:17, 2] = 1.0
    mk[12:14, 3] = 1.0
    com["mskp"] = bf(mk)
    mkT = np.zeros((3, 17), f32)
    mkT[0, 0:4] = 1.0
    mkT[1, 4:12] = 1.0
    mkT[2, 14:17] = 1.0
    com["mskTp"] = bf(mkT)
    wsrc["tw1"] = ip["tn_w1"][:, :D, :]
    wsrc["tw1t"] = np.ascontiguousarray(ip["tn_w1"][:, D:D + 17, :])
    wsrc["tw2"] = ip["tn_w2"]
    wsrc["fw2"] = ip["ff_w2"]

    def pack_w(specs, cm, ncols):
        pk = np.zeros((L, 128, ncols), f32)
        for name, kdim, ndim, bnd in specs:
            arr = np.asarray(wsrc[name], f32)
            for ci, (c0, c1) in enumerate(bnd or chunks(kdim)):
                off = cm[(name, ci)]
                pk[:, : c1 - c0, off:off + ndim] = arr[:, c0:c1, :]
        return pk

    pkA = pack_w(WPK_A, WPKA_CM, WPKA_COLS)
    # v bias + softmax-sum one-slots ride contraction row 64 of the wv chunk-2 tile
    offv = WPKA_CM[("wv", 2)]
    pkA[:, 64, offv:offv + HP] = bvp
    com["wpackA"] = bf(pkA)
    com["wpackM"] = bf(pack_w(WPK_M, WPKM_CM, WPKM_COLS))
    taps = np.concatenate([ip["ck1"].reshape(1, D, D), ip["ck3"].reshape(9, D, D),
                           ip["ck5"].reshape(25, D, D), ip["ck7"].reshape(49, D, D)], axis=0)
    com["ck"] = bf(taps.reshape(NTAPG, 2, D, D).transpose(0, 2, 1, 3).reshape(NTAPG, D, 2 * D))
    com["fusw"] = bf(ip["fus_w"])
    com["ow1"], com["ow2"], com["ow3"] = bf(ip["op_w1"]), bf(ip["op_w2"]), bf(ip["op_w3"])

    # packed per-partition vectors
    vec_src = {}
    for l in range(L):
        vec_src[("bqs", l)] = bqp[l]
        vec_src[("bk", l)] = bkp[l]
        vec_src[("bo", l)] = ip["bo"][l]
        vec_src[("lag", l)] = ip["ln_a_g"][l]
        vec_src[("lab", l)] = ip["ln_a_b"][l]
        vec_src[("l2g", l)] = ip["ln2_g"][l]
        vec_src[("l2b", l)] = ip["ln2_b"][l]
        vec_src[("tb1", l)] = ip["tn_b1"][l]
        vec_src[("tb2", l)] = ip["tn_b2"][l]
        vec_src[("tb3", l)] = ip["tn_b3"][l] * 0.3
        vec_src[("fb1", l)] = ip["ff_b1"][l]
        vec_src[("fb2", l)] = ip["ff_b2"][l]
    vec_src[("inb", None)] = ip["in_emb_b"]
    for i, kk in enumerate((1, 3, 5, 7)):
        vec_src[(f"cb{i}", None)] = ip[f"cb{kk}"]
    vec_src[("fusb", None)] = ip["fus_b"]
    vec_src[("ob1", None)] = ip["op_b1"]
    vec_src[("ob2", None)] = ip["op_b2"]
    vec_src[("ob3", None)] = ip["op_b3"]
    vecpack = np.zeros((128, VEC_NCOL), f32)
    for (name, l, ci), col in VEC_COLMAP.items():
        src = np.asarray(vec_src[(name, l)], f32)
        c0, c1 = chunks(len(src))[ci]
        vecpack[: c1 - c0, col] = src[c0:c1]
    com["vecpack"] = vecpack

    # geometric bias, transposed + kchunk-padded + head-pair interleaved:
    # [l, kchunk, g2, r, 2*q]; per core, key rows are reordered [own | peer]
    dist_idx, dir_idx = ip["dist_idx"], ip["dir_idx"]
    bhkq_l = []
    for l in range(L):
        bqk = ip["dist_emb"][l][dist_idx] + ip["dir_emb"][l][dir_idx]   # [q, k, h] f32
        bhkq_l.append(np.ascontiguousarray(bqk.transpose(2, 1, 0)) * BIAS_SCALE)  # [h, k, q]
    bias_half = []
    f8 = None
    import ml_dtypes as _md
    f8 = _md.float8_e4m3fn
    for half in range(2):
        own = slice(SH * half, SH * half + SH)
        peer_s = slice(SH * (1 - half), SH * (1 - half) + SH)
        bt = np.zeros((L, len(KCH), 128, NH // 2, 2 * SH), dtype=f8)
        for l in range(L):
            ordered = np.concatenate([bhkq_l[l][:, own, own], bhkq_l[l][:, peer_s, own]], axis=1)
            for ci, (k0, k1) in enumerate(KCH):
                for g2 in range(NH // 2):
                    bt[l, ci, : k1 - k0, g2, 0:SH] = ordered[2 * g2, k0:k1, :].astype(f8)
                    bt[l, ci, : k1 - k0, g2, SH:2 * SH] = ordered[2 * g2 + 1, k0:k1, :].astype(f8)
        bias_half.append(bt)

    peT_full = np.ascontiguousarray(ip["pe"].reshape(S, D).T.astype(f32))  # [D, S]
    grids = ip["input_grid"].reshape(B, S)

    in_maps = []
    for c in range(8):
        b, half = c // 2, c % 2
        t0, t1 = SH * half, SH * (1 - half)
        m = dict(com)
        m["grid"] = np.concatenate([grids[b, t0:t0 + SH], grids[b, t1:t1 + SH]]
                                   ).astype(f32).reshape(1, S).astype(bf16)
        m["peT"] = bf(np.concatenate([peT_full[:, t0:t0 + SH], peT_full[:, t1:t1 + SH]], axis=1))
        m["biasT"] = bias_half[half]
        in_maps.append(m)
    return in_maps


_BUILT = None


def _fuse_ldweights(nc):
    """Drop tile_legalize's explicit InstLdweights (the paired InstMatmult is
    still self-loading); keep their sync waits/updates on EventSemaphores so
    walrus can compile with --enable-ldw-opt=true and background the loads."""
    for f in nc.m.functions:
        for bb in f.blocks:
            il = bb.instructions
            newlist = []
            changed = False
            for i, ins in enumerate(il):
                if type(ins).__name__ == "InstLdweights":
                    changed = True
                    if i + 1 < len(il) and type(il[i + 1]).__name__ == "InstMatmult":
                        il[i + 1].ldweights = True   # matmul self-loads now
                    si = ins.sync_info
                    nw = len(si.on_wait) if si else 0
                    nu = len(si.on_update) if si else 0
                    if nw == 0 and nu == 0:
                        continue
                    ev = mybir.InstEventSemaphore(
                        name=f"ldwev_{ins.name}", engine=ins.engine,
                        ins=[], outs=[], sync_info=si, debug=ins.debug)
                    newlist.append(ev)
                    continue
                newlist.append(ins)
            if changed:
                bb.instructions = newlist


def get_built():
    global _BUILT
    if _BUILT is None:
        import os
        nc = bacc.Bacc("TRN2", target_bir_lowering=False, num_devices=8)
        build(nc)
        nc.finalize()
        if os.environ.get("ATLAS_LDWFUSE") == "1":
            _fuse_ldweights(nc)
        _BUILT = nc
    return _BUILT


_LDW_PATCHED = False


def _enable_ldw_opt():
    """Compile NEFFs with --enable-ldw-opt=true (overlaps LDWEIGHTS with matmuls)."""
    global _LDW_PATCHED
    if _LDW_PATCHED:
        return
    import concourse.bass_utils as bu
    orig = bu.run_command

    def patched(cmd, cwd=None, **kw):
        cmd = ["--enable-ldw-opt=true" if c == "--enable-ldw-opt=false" else c for c in cmd]
        return orig(cmd, cwd=cwd, **kw)

    bu.run_command = patched
    _LDW_PATCHED = True


def kernel(**inputs):
    from concourse.bass_utils import run_bass_kernel_spmd
    import os
    if os.environ.get("ATLAS_LDWOPT") == "1":
        _enable_ldw_opt()
    nc = get_built()
    in_maps = prep_inputs(inputs)
    trace = bool(os.environ.get("ATLAS_TRACE"))
    res = run_bass_kernel_spmd(nc, in_maps, core_ids=list(range(8)), trace=trace)
    if trace:
        kernel.last_exec_time_ns = res.exec_time_ns
        kernel.last_results = res
    out = np.zeros((B, G, G, 10), np.float32)
    for c in range(8):
        b, half = c // 2, c % 2
        out[b, 15 * half:15 * half + 15] = res.results[c]["out"].T.reshape(15, G, 10)
    return out


# revision 50
# speedup vs baseline: 1.0109x; 1.0013x over previous
"""AtlasV4Transformer Trainium2 kernel — 8-core SPMD, token-split data parallel.

Sharding: core c -> batch b = c//2, token half = c%2 (450 of 900 grid tokens).
Activations are feature-major on chip: x^T [D(partitions, 3 chunk tiles), tokens].
Attention uses transposed scores S^T[k,q] per head; softmax row sums ride along
the AV matmul via a constant-1 slot built into the head-padded V layout (head h
occupies a 64-wide slot: [1 | v(40) | 0]).  Head pairs are processed jointly:
the two score matmuls of a pair row-pack the PE array (contraction rows 0:40 /
64:104) into one 2-bank PSUM tile, bias-add runs as one DVE pass and exp as one
ACT pass over [kc, 900].  The geometric-transform AllReduce feeds a rank-17
bias column (cvec) folded into tn1's gelu bias, so the tn MLP matmuls never
wait on the collective.  The per-head geometric bias table gather is
materialized on the host and streamed as a pre-interleaved fp8 input.
"""
import sys

import numpy as np

if "/opt/trn_rl_repo" not in sys.path:
    sys.path.insert(0, "/opt/trn_rl_repo")

import concourse.bass as bass
import concourse.bacc as bacc
import concourse.mybir as mybir
from concourse import tile

F32 = mybir.dt.float32
BF16 = mybir.dt.bfloat16
F8 = mybir.dt.float8e4
BIAS_SCALE = 64.0
AF = mybir.ActivationFunctionType
OP = mybir.AluOpType

B, G, D, L, NH, DK, S = 4, 30, 320, 4, 8, 40, 900
SH = S // 2            # tokens owned per core
FFD = 4 * D            # 1280
HP = 512               # head-padded q/k/v width (8 heads x 64)
SCALE = 1.0 / np.sqrt(DK)
EPS = 1e-5
PAIRS = [[0, 1], [2, 3], [4, 5], [6, 7]]
PG = G + 6             # padded grid 36
NTAPG = 42             # conv taps packed 2 per group (84 total)


def chunks(n, c=128):
    return [(i, min(i + c, n)) for i in range(0, n, c)]


DCH = chunks(D)          # 3 feature chunks
# key-token chunks aligned to the 450/450 own|peer split
KCH = chunks(SH) + [(SH + a, SH + b) for (a, b) in chunks(SH)]

# packed per-layer weight blocks: fixed column order shared by host and device
TB1 = [(0, 128), (128, 256), (256, 320)]
WPK_A = [("wq", D, HP, None), ("wk", D, HP, None), ("wv", D, HP, None), ("wo", HP, D, None)]
WPK_M = [("wcat", D, 17, None), ("tw1", D, 640, TB1), ("tw1t", 17, 640, [(0, 17)]),
         ("tw2", 640, D, None), ("tw3", D, D, None), ("fw1", D, FFD, None), ("fw2", FFD, D, None)]


def build_wpk_colmap(specs):
    cm, col = {}, 0
    for name, kdim, ndim, bnd in specs:
        for ci, _ in enumerate(bnd or chunks(kdim)):
            cm[(name, ci)] = col
            col += ndim
    return cm, col


WPKA_CM, WPKA_COLS = build_wpk_colmap(WPK_A)
WPKM_CM, WPKM_COLS = build_wpk_colmap(WPK_M)


def wpk_slices(t, specs, cm):
    out = {}
    for name, kdim, ndim, bnd in specs:
        out[name] = [t[:, cm[(name, ci)]:cm[(name, ci)] + ndim]
                     for ci, _ in enumerate(bnd or chunks(kdim))]
    return out


# packed per-partition vectors: fixed column order shared by host and device
VEC_LAYER_SPECS = [("bqs", HP), ("bk", HP), ("bo", D), ("lag", D), ("lab", D),
                   ("l2g", D), ("l2b", D), ("tb1", 640), ("tb2", D), ("tb3", D),
                   ("fb1", FFD), ("fb2", D)]
VEC_GLOBAL_SPECS = [("inb", D), ("cb0", D), ("cb1", D), ("cb2", D), ("cb3", D),
                    ("fusb", D), ("ob1", 160), ("ob2", 80), ("ob3", 10)]


def build_vec_colmap():
    cm = {}
    col = 0
    for l in range(L):
        for name, n in VEC_LAYER_SPECS:
            for ci in range(len(chunks(n))):
                cm[(name, l, ci)] = col
                col += 1
    for name, n in VEC_GLOBAL_SPECS:
        for ci in range(len(chunks(n))):
            cm[(name, None, ci)] = col
            col += 1
    return cm, col


VEC_COLMAP, VEC_NCOL = build_vec_colmap()


def build(nc):
    dpi = lambda name, shape, dt: nc.declare_dram_parameter(name, list(shape), dt, isOutput=False)

    P = {}
    P["grid"] = dpi("grid", [1, S], BF16)
    P["iota10"] = dpi("iota10", [10, 1], F32)
    P["peT"] = dpi("peT", [D, S], BF16)
    P["inw"] = dpi("inw", [10, D], BF16)
    P["vecpack"] = dpi("vecpack", [128, VEC_NCOL], F32)
    P["wpackA"] = dpi("wpackA", [L, 128, WPKA_COLS], BF16)
    P["wpackM"] = dpi("wpackM", [L, 128, WPKM_COLS], BF16)
    P["bcat"] = dpi("bcat", [L, 1, 17], BF16)
    P["mskp"] = dpi("mskp", [17, 4], BF16)
    P["mskTp"] = dpi("mskTp", [3, 17], BF16)
    # bias, transposed + chunk-padded + host-interleaved: [l, kchunk, r(128), g2, 2*q(450)]
    # (g2 innermost so one contiguous DMA per (kchunk, gp) covers both heads of both j's)
    P["biasT"] = dpi("biasT", [L, len(KCH), 128, NH // 2, 2 * SH], F8)
    P["ck"] = dpi("ck", [NTAPG, D, 2 * D], BF16)
    P["fusw"] = dpi("fusw", [FFD, D], BF16)
    P["ow1"] = dpi("ow1", [D, 160], BF16)
    P["ow2"] = dpi("ow2", [160, 80], BF16)
    P["ow3"] = dpi("ow3", [80, 10], BF16)
    P["out"] = nc.declare_dram_parameter("out", [10, SH], F32, isOutput=True)

    with tile.TileContext(nc) as tc:
        with (
            tc.tile_pool(name="const", bufs=1) as cp,
            tc.tile_pool(name="wts", bufs=1) as wp,
            tc.tile_pool(name="acts", bufs=1) as ap_,
            tc.tile_pool(name="tmp", bufs=1) as tp,
            tc.tile_pool(name="psum", bufs=1, space="PSUM") as pp,
            tc.tile_pool(name="dram", bufs=1, space="DRAM") as dram,
        ):
            build_body(nc, tc, cp, wp, ap_, tp, pp, dram, P)
    return nc


def load_w(nc, wp, param, l, kdim, ndim, name, bufs=1, boundaries=None):
    ts = []
    for ci, (c0, c1) in enumerate(boundaries or chunks(kdim)):
        t = wp.tile([128, ndim], BF16, tag=f"{name}{ci}", bufs=bufs, name=f"{name}{ci}")
        src = param[l, c0:c1, :] if l is not None else param[c0:c1, :]
        nc.sync.dma_start(out=t[: c1 - c0, :], in_=src)
        ts.append(t)
    return ts


def ps_work(pp, name="ps_w"):
    """One 2-bank [128, 1024] f32 PSUM tile from the shared rotation."""
    return pp.tile([128, 1024], F32, tag="ps_w", bufs=3, name=name)


def warm_keep(nc, ones, trash, n):
    """Dependency-free filler matmuls into a throwaway PSUM row.  Emitted into
    known PE-idle windows (collective wait, conv pad construction) so the HAM
    activity monitor keeps the array at the 2.4 GHz clock state."""
    for _ in range(n):
        nc.tensor.matmul(trash, ones[0:128, 0:1], ones[:, 0:SH], start=True, stop=True)


def mm_proj(nc, pp, Wt, X, kdim, ndim, evict):
    kch = chunks(kdim)
    for ni, (n0, n1) in enumerate(chunks(ndim)):
        ps = ps_work(pp, "ps_mm")
        for ci, (c0, c1) in enumerate(kch):
            nc.tensor.matmul(ps[: n1 - n0, 0:SH], Wt[ci][: c1 - c0, n0:n1], X[ci][: c1 - c0, :],
                             start=(ci == 0), stop=(ci == len(kch) - 1))
        evict(ps, ni, n0, n1)


def build_body(nc, tc, cp, wp, ap_, tp, pp, dram, P):
    # ---------------- constants ----------------
    ones = cp.tile([128, SH], BF16, tag="ones", bufs=1, name="ones")
    nc.vector.memset(ones[:], 1.0)
    ones_f = cp.tile([1, 128], F32, tag="ones_f", bufs=1, name="ones_f")
    nc.vector.memset(ones_f[:], 1.0)
    # softmax group masks for the 17-wide geometric transform column (host consts)
    msk = cp.tile([17, 4], BF16, tag="msk", bufs=1, name="msk")
    nc.sync.dma_start(out=msk[:], in_=P["mskp"][:])
    mskT = cp.tile([3, 17], BF16, tag="mskT", bufs=1, name="mskT")
    nc.sync.dma_start(out=mskT[:], in_=P["mskTp"][:])

    eps_t = cp.tile([1, 1], F32, tag="eps", bufs=1, name="eps_t")
    nc.vector.memset(eps_t[:], EPS)
    iota_t = cp.tile([10, 1], F32, tag="iota", bufs=1, name="iota_t")
    nc.sync.dma_start(out=iota_t[:], in_=P["iota10"][:])
    peT_t = [cp.tile([128, S], BF16, tag=f"peT{ci}", bufs=1, name=f"peT{ci}") for ci in range(3)]
    for ci, (c0, c1) in enumerate(DCH):
        nc.sync.dma_start(out=peT_t[ci][: c1 - c0, :], in_=P["peT"][c0:c1, :])
    grid_t = cp.tile([1, S], BF16, tag="grid", bufs=1, name="grid_t")
    nc.sync.dma_start(out=grid_t[:], in_=P["grid"][:])
    inw_t = cp.tile([10, D], BF16, tag="inw", bufs=1, name="inw_t")
    nc.sync.dma_start(out=inw_t[:], in_=P["inw"][:])
    vp = cp.tile([128, VEC_NCOL], F32, tag="vecpack", bufs=1, name="vp")
    nc.sync.dma_start(out=vp[:], in_=P["vecpack"][:])

    def vec_aps(name, n, l=None):
        return [vp[: c1 - c0, VEC_COLMAP[(name, l, ci)]:VEC_COLMAP[(name, l, ci)] + 1]
                for ci, (c0, c1) in enumerate(chunks(n))]

    def vec_wide(name, l, n):
        col0 = VEC_COLMAP[(name, l, 0)]
        return vp[:, col0:col0 + n]

    bcat_t = []
    for l in range(L):
        t2_ = cp.tile([1, 17], BF16, tag=f"bcat{l}", bufs=1, name=f"bcat{l}")
        nc.sync.dma_start(out=t2_[:], in_=P["bcat"][l])
        bcat_t.append(t2_)

    # ---------------- embedding (both halves; kills the layer-0 gather) ----------------
    oh = tp.tile([10, S], BF16, tag="oh", bufs=1, name="oh")
    for half in range(2):
        hs = slice(SH * half, SH * half + SH)
        ps_g = ps_work(pp, "ps_g")
        nc.tensor.matmul(ps_g[:10, 0:SH], ones[0:1, 0:10], grid_t[0:1, hs], start=True, stop=True)
        nc.vector.tensor_scalar(out=oh[:, hs], in0=ps_g[:10, 0:SH], scalar1=iota_t[:10, :],
                                scalar2=None, op0=OP.is_equal)

    inb_c = vec_aps("inb", D)
    xs = [ap_.tile([128, SH], BF16, tag=f"xs{ci}", bufs=1, name=f"xs{ci}") for ci in range(3)]
    xp0 = [ap_.tile([128, SH], BF16, tag=f"xp{ci}", bufs=1, name=f"xp{ci}") for ci in range(3)]
    for ci, (c0, c1) in enumerate(DCH):
        for half in range(2):
            hs = slice(SH * half, SH * half + SH)
            dst = xs[ci] if half == 0 else xp0[ci]
            pse = ps_work(pp, "pse")
            nc.tensor.matmul(pse[: c1 - c0, 0:SH], inw_t[:, c0:c1], oh[:, hs], start=True, stop=True)
            nc.vector.scalar_tensor_tensor(
                out=dst[: c1 - c0, :], in0=pse[: c1 - c0, 0:SH], scalar=inb_c[ci],
                in1=peT_t[ci][: c1 - c0, hs], op0=OP.add, op1=OP.add)
    # ones row for the v-bias contraction trick
    nc.vector.memset(xs[2][64:65, :], 1.0)
    nc.vector.memset(xp0[2][64:65, :], 1.0)

    # ---------------- transformer layers ----------------
    W = load_attn_w(nc, wp, P, 0)
    for l in range(L):
        Wnext = {} if l + 1 < L else None
        xs = layer(nc, wp, ap_, tp, pp, dram, P, l, xs, vec_aps, vec_wide, bcat_t[l],
                   ones, ones_f, msk, mskT, eps_t, xp0 if l == 0 else None, W, Wnext)
        W = Wnext

    # ---------------- conv fusion + head ----------------
    conv_head(nc, cp, wp, ap_, tp, pp, dram, P, xs, vec_aps, ones, ones_f)


def load_attn_w(nc, wp, P, l):
    """q/k/v/o weights as ONE packed DMA, prefetched one layer ahead (bufs=2)."""
    t = wp.tile([128, WPKA_COLS], BF16, tag="wpkA", bufs=2, name="wpkA")
    nc.sync.dma_start(out=t[:], in_=P["wpackA"][l])
    return wpk_slices(t, WPK_A, WPKA_CM)


def layer(nc, wp, ap_, tp, pp, dram, P, l, xs, vec_aps, vec_wide, bcat_t, ones, ones_f,
          msk, mskT, eps_t, xp0, W, Wnext):
    bqs_c = vec_aps("bqs", HP, l)
    bk_c = vec_aps("bk", HP, l)
    bo_c = vec_aps("bo", D, l)
    lag_c = vec_aps("lag", D, l)
    lab_c = vec_aps("lab", D, l)
    l2g_c = vec_aps("l2g", D, l)
    l2b_c = vec_aps("l2b", D, l)
    tb2_c = vec_aps("tb2", D, l)
    tb3_c = vec_aps("tb3", D, l)
    fb1_c = vec_aps("fb1", FFD, l)
    fb2_c = vec_aps("fb2", D, l)
    tb1_w = vec_wide("tb1", l, 5)

    wq_t, wk_t, wv_t, wo_t = W["wq"], W["wk"], W["wv"], W["wo"]
    # MLP weights: one packed DMA issued at layer top, lands under attention
    wm = wp.tile([128, WPKM_COLS], BF16, tag="wpkM", bufs=1, name="wpkM")
    nc.sync.dma_start(out=wm[:], in_=P["wpackM"][l])
    WM = wpk_slices(wm, WPK_M, WPKM_CM)
    wcat_t, tw1_t, tw1t_t, tw2_t = WM["wcat"], WM["tw1"], WM["tw1t"][0], WM["tw2"]
    tw3_t, fw1_t, fw2_t = WM["tw3"], WM["fw1"], WM["fw2"]
    if Wnext is not None:
        Wnext.update(load_attn_w(nc, wp, P, l + 1))

    # ---- q projection, scaled; head-padded rows [64h, 64h+40) ----
    qp = [ap_.tile([128, SH], BF16, tag=f"qp{g}", bufs=1, name=f"qp{g}") for g in range(4)]

    def evict_q(ps, ni, n0, n1):
        nc.vector.tensor_scalar(out=qp[ni][: n1 - n0, :], in0=ps[: n1 - n0, 0:SH],
                                scalar1=SCALE, scalar2=bqs_c[ni], op0=OP.mult, op1=OP.add)

    mm_proj(nc, pp, wq_t, xs, D, HP, evict_q)

    # ---- peer-x gather (single collective; layer 0 has xp precomputed) ----
    if xp0 is not None:
        xp = xp0
    else:
        xgin = dram.tile([D, SH], BF16, tag="xgin", bufs=2, name="xgin")
        xgout = dram.tile([2, D, SH], BF16, tag="xgout", bufs=2, name="xgout")
        for ci, (c0, c1) in enumerate(DCH):
            nc.sync.dma_start(out=xgin[c0:c1, :], in_=xs[ci][: c1 - c0, :])
        nc.gpsimd.collective_compute("AllGather", OP.bypass, replica_groups=PAIRS,
                                     ins=[xgin[:].opt()], outs=[xgout[:].opt()])
        peer = (nc.sync.partition_id() + 1) % 2
        xgout_f = xgout[:].rearrange("g p q -> (g p) q")
        xp = [ap_.tile([128, SH], BF16, tag=f"xp{ci}", bufs=1, name=f"xp{ci}") for ci in range(3)]
        for ci, (c0, c1) in enumerate(DCH):
            nc.sync.dma_start(out=xp[ci][: c1 - c0, :],
                              in_=xgout_f[bass.ds(peer * D + c0, c1 - c0), :])
        nc.vector.memset(xp[2][64:65, :], 1.0)

    # ---- k for all 900 keys, feature-major [own cols | peer cols] ----
    khp = [ap_.tile([128, S], BF16, tag=f"khp{g2}", bufs=1, name=f"khp{g2}") for g2 in range(4)]

    def evict_k_own(ps, ni, n0, n1):
        nc.vector.tensor_scalar(out=khp[ni][: n1 - n0, 0:SH], in0=ps[: n1 - n0, 0:SH],
                                scalar1=bk_c[ni], scalar2=None, op0=OP.add)

    mm_proj(nc, pp, wk_t, xs, D, HP, evict_k_own)

    def evict_k_peer(ps, ni, n0, n1):
        nc.vector.tensor_scalar(out=khp[ni][: n1 - n0, SH:S], in0=ps[: n1 - n0, 0:SH],
                                scalar1=bk_c[ni], scalar2=None, op0=OP.add)

    # ---- v, token-major rows [own | peer], 64-wide head slots; the v bias and
    # the softmax-sum 1-slots ride contraction row 64 of chunk 2 (ones row) ----
    def v_chunks(rng):
        for si in rng:
            k0, k1 = KCH[si]
            kc = k1 - k0
            src_x, off = (xs, 0) if k1 <= SH else (xp, SH)
            psv = ps_work(pp, "psv")
            for ci, (c0, c1) in enumerate(DCH):
                kk = (c1 - c0) + (1 if ci == 2 else 0)
                nc.tensor.matmul(psv[:kc, 0:HP], src_x[ci][:kk, k0 - off:k1 - off],
                                 wv_t[ci][:kk, :], start=(ci == 0), stop=(ci == 2))
            t = ap_.tile([128, HP], BF16, tag=f"va{si}", bufs=1, name=f"va{si}")
            nc.vector.tensor_copy(t[:kc, :], psv[:kc, 0:HP])
            va.append(t)

    va = []
    v_chunks(range(4))
    mm_proj(nc, pp, wk_t, xp, D, HP, evict_k_peer)
    v_chunks(range(4, 8))

    # ---- attention: head pairs processed jointly; all own-key chunks (ci<4)
    # run before any peer-key dependency, hiding the x-gather latency ----
    attnT = [ap_.tile([128, SH], BF16, tag=f"at{g}", bufs=2, name=f"at{g}") for g in range(4)]
    for gp in range(2):
        ps_avs = [pp.tile([128, SH], F32, tag=f"ps_av{j}", bufs=1, name=f"ps_av{j}")
                  for j in range(2)]

        def emit_av(j, ci, kc, ee):
            # col-packed AV pair: the two heads use disjoint 32-col groups
            a0 = 128 * (2 * gp + j)
            nc.tensor.matmul(ps_avs[j][0:64, :], va[ci][:kc, a0:a0 + 64],
                             ee[:kc, 0:SH], start=(ci == 0), stop=(ci == len(KCH) - 1),
                             tile_position=(0, 0))
            nc.tensor.matmul(ps_avs[j][64:128, :], va[ci][:kc, a0 + 64:a0 + 128],
                             ee[:kc, SH:2 * SH], start=(ci == 0), stop=(ci == len(KCH) - 1),
                             tile_position=(0, 64))

        pend = []   # AV pairs lag their scores by 1 unit
        for ci, (k0, k1) in enumerate(KCH):
            kc = k1 - k0
            btp = tp.tile([128, 4 * SH], F8, tag="bias", bufs=2, name="btp")
            nc.gpsimd.dma_start(
                out=btp[:kc, :],
                in_=P["biasT"][l, ci, :kc, 2 * gp:2 * gp + 2, :].rearrange("p g q -> p (g q)"))
            for j in range(2):
                g2 = 2 * gp + j
                bt2 = btp[:, 2 * SH * j:2 * SH * j + 2 * SH]
                psp = ps_work(pp, "psp")
                # row-packed score pair: contraction rows 0:40 and 64:104 run
                # concurrently in disjoint 32-row groups of the PE array
                nc.tensor.matmul(psp[:kc, 0:SH], khp[g2][0:40, k0:k1], qp[g2][0:40, :],
                                 start=True, stop=True, tile_position=(0, 0))
                nc.tensor.matmul(psp[:kc, 512:512 + SH], khp[g2][64:104, k0:k1],
                                 qp[g2][64:104, :], start=True, stop=True,
                                 tile_position=(64, 0))
                es = tp.tile([128, 2 * SH], BF16, tag="esc", bufs=2, name="es")
                nc.vector.scalar_tensor_tensor(
                    out=es[:kc].rearrange("p (h q) -> p h q", h=2),
                    in0=bt2[:kc].rearrange("p (h q) -> p h q", h=2),
                    scalar=1.0 / BIAS_SCALE,
                    in1=psp[:kc].rearrange("p (h q) -> p h q", h=2)[:, :, 0:SH],
                    op0=OP.mult, op1=OP.add)
                ee = tp.tile([128, 2 * SH], BF16, tag="eexp", bufs=2, name="ee")
                nc.scalar.activation(ee[:kc, :], es[:kc, :], AF.Exp)
                pend.append((j, ci, kc, ee))
                if len(pend) > 1:
                    emit_av(*pend.pop(0))
        for u in pend:
            emit_av(*u)
        for j in range(2):
            g2 = 2 * gp + j
            ps_av = ps_avs[j]
            # sum rows 0 / 64 -> SBUF rows, broadcast to partition halves via
            # matmul, then reciprocal runs partition-parallel on [128, SH]
            s2a = tp.tile([1, SH], BF16, tag="rec", bufs=2, name="s2a")
            s2b = tp.tile([1, SH], BF16, tag="recb", bufs=2, name="s2b")
            nc.vector.tensor_copy(s2a[:], ps_av[0:1, :])
            nc.vector.tensor_copy(s2b[:], ps_av[64:65, :])
            ps_bc = ps_work(pp, "ps_bc")
            nc.tensor.matmul(ps_bc[0:64, 0:SH], ones[0:1, 0:64], s2a[:], start=True, stop=True)
            nc.tensor.matmul(ps_bc[64:128, 0:SH], ones[0:1, 0:64], s2b[:], start=True, stop=True)
            bc = tp.tile([128, SH], F32, tag="bcn", bufs=1, name="bc")
            nc.vector.reciprocal_approx_fast(bc[:], ps_bc[:, 0:SH])
            nc.vector.tensor_tensor(out=attnT[g2][:], in0=ps_av[:], in1=bc[:], op=OP.mult)

    # ---- wo projection + residual + LN ----
    res = [tp.tile([128, SH], BF16, tag=f"res{ci}", bufs=2, name=f"res{ci}") for ci in range(3)]

    def evict_o(ps, ni, n0, n1):
        nc.vector.scalar_tensor_tensor(out=res[ni][: n1 - n0, :], in0=ps[: n1 - n0, 0:SH],
                                       scalar=bo_c[ni], in1=xs[ni][: n1 - n0, :],
                                       op0=OP.add, op1=OP.add)

    mm_proj(nc, pp, wo_t, attnT, HP, D, evict_o)
    xs1 = layernorm(nc, ap_, tp, pp, res, lag_c, lab_c, ones, ones_f, eps_t, "xsa")

    # ---- geometric transform: pair all-reduce -> 17-wide transform params ->
    # rank-17 contribution becomes a per-partition bias column for tn1 ----
    gin = dram.tile([128, 3], F32, tag="gin", bufs=2, name="gin")
    gout = dram.tile([128, 3], F32, tag="gout", bufs=2, name="gout")
    gred = tp.tile([128, 3], F32, tag="gred", bufs=2, name="gred")
    for ci, (c0, c1) in enumerate(DCH):
        nc.vector.reduce_sum(gred[: c1 - c0, ci:ci + 1], xs1[ci][: c1 - c0, :],
                             axis=mybir.AxisListType.X)
    nc.sync.dma_start(out=gin[:], in_=gred[:])
    nc.gpsimd.collective_compute("AllReduce", OP.add, replica_groups=PAIRS,
                                 ins=[gin[:].opt()], outs=[gout[:].opt()])
    gf = tp.tile([128, 3], F32, tag="gf", bufs=2, name="gf")
    nc.sync.dma_start(out=gf[:], in_=gout[:])
    gbf3 = tp.tile([128, 3], BF16, tag="gbf3", bufs=2, name="gbf3")
    nc.vector.tensor_copy(gbf3[:], gf[:])

    # ---- tn1 x-part matmuls for the first 3 chunks, emitted BEFORE the
    # collective-dependent geo matmuls so the in-order PE queue overlaps
    # them with the all-reduce latency ----
    t1 = [tp.tile([128, SH], BF16, tag=f"t1_{ni}", bufs=1, name=f"t1_{ni}") for ni in range(5)]
    t1ps = []
    for ni, (n0, n1) in list(enumerate(chunks(640)))[:3]:
        ps = ps_work(pp, "ps_t1")
        nc.tensor.matmul(ps[: n1 - n0, 0:SH], tw1_t[0][:128, n0:n1], xs1[0][:128, :], start=True, stop=False)
        nc.tensor.matmul(ps[: n1 - n0, 0:SH], tw1_t[1][:128, n0:n1], xs1[1][:128, :], start=False, stop=False)
        nc.tensor.matmul(ps[: n1 - n0, 0:SH], tw1_t[2][:64, n0:n1], xs1[2][:64, :], start=False, stop=True)
        t1ps.append(ps)

    # tp column [17,1]: wcat^T @ g + bcat
    gps_a = pp.tile([128, SH], F32, tag="ps_av0", bufs=1, name="gps_a")
    gps_b = pp.tile([128, SH], F32, tag="ps_av1", bufs=1, name="gps_b")
    warm_keep(nc, ones, gps_b[32:33, 0:SH], 56)
    for ci, (c0, c1) in enumerate(DCH):
        nc.tensor.matmul(gps_a[0:17, 0:1], wcat_t[ci][: c1 - c0, :], gbf3[: c1 - c0, ci:ci + 1],
                         start=(ci == 0), stop=False)
    nc.tensor.matmul(gps_a[0:17, 0:1], bcat_t[:], ones[0:1, 0:1], start=False, stop=True)
    # softmax groups [0:4),[4:12),[14:17); tanh [12:14) — all on the column
    exc = tp.tile([128, 1], BF16, tag="exc", bufs=2, name="exc")
    nc.scalar.activation(exc[0:17, :], gps_a[0:17, 0:1], AF.Exp)
    nc.tensor.matmul(gps_b[0:3, 0:1], msk[:, 0:3], exc[0:17, :], start=True, stop=True)
    rg = tp.tile([3, 1], F32, tag="rg", bufs=2, name="rg")
    nc.vector.reciprocal(rg[:], gps_b[0:3, 0:1])
    rgb = tp.tile([3, 1], BF16, tag="rgb", bufs=2, name="rgb")
    nc.vector.tensor_copy(rgb[:], rg[:])
    nc.tensor.matmul(gps_b[0:17, 4:5], mskT[:, :], rgb[:], start=True, stop=True)
    # softmax part (rows 12:14 scale to 0) + tanh part merged via the mask column
    tps = tp.tile([128, 1], BF16, tag="tps", bufs=2, name="tps")
    nc.vector.tensor_tensor(out=tps[0:17, :], in0=exc[0:17, :], in1=gps_b[0:17, 4:5], op=OP.mult)
    tha = tp.tile([128, 1], BF16, tag="tha", bufs=2, name="tha")
    nc.scalar.activation(tha[0:17, :], gps_a[0:17, 0:1], AF.Tanh)
    tpc = tp.tile([128, 1], BF16, tag="tpc", bufs=2, name="tpc")
    nc.vector.scalar_tensor_tensor(out=tpc[0:17, :], in0=tha[0:17, :], scalar=msk[0:17, 3:4],
                                   in1=tps[0:17, :], op0=OP.mult, op1=OP.add)
    # cvec[640] = tw1_tp^T @ tp  (5 chunk columns) + tb1 -> effective t1 bias
    for ni, (n0, n1) in enumerate(chunks(640)):
        nc.tensor.matmul(gps_a[: n1 - n0, 16 + ni:17 + ni], tw1t_t[0:17, n0:n1], tpc[0:17, :],
                         start=True, stop=True)
    t1b = tp.tile([128, 5], F32, tag="t1b", bufs=2, name="t1b")
    nc.vector.tensor_tensor(out=t1b[:], in0=gps_a[:, 16:21], in1=tb1_w, op=OP.add)

    # ---- tn1 evictions + remaining chunks ----
    for ni, (n0, n1) in list(enumerate(chunks(640)))[:3]:
        nc.scalar.activation(t1[ni][: n1 - n0, :], t1ps[ni][: n1 - n0, 0:SH], AF.Gelu,
                             bias=t1b[: n1 - n0, ni:ni + 1], scale=1.0)
    for ni, (n0, n1) in list(enumerate(chunks(640)))[3:]:
        ps = ps_work(pp, "ps_t1")
        nc.tensor.matmul(ps[: n1 - n0, 0:SH], tw1_t[0][:128, n0:n1], xs1[0][:128, :], start=True, stop=False)
        nc.tensor.matmul(ps[: n1 - n0, 0:SH], tw1_t[1][:128, n0:n1], xs1[1][:128, :], start=False, stop=False)
        nc.tensor.matmul(ps[: n1 - n0, 0:SH], tw1_t[2][:64, n0:n1], xs1[2][:64, :], start=False, stop=True)
        nc.scalar.activation(t1[ni][: n1 - n0, :], ps[: n1 - n0, 0:SH], AF.Gelu,
                             bias=t1b[: n1 - n0, ni:ni + 1], scale=1.0)
    t2 = [tp.tile([128, SH], BF16, tag=f"t2_{ni}", bufs=1, name=f"t2_{ni}") for ni in range(3)]

    def evict_t2(ps, ni, n0, n1):
        nc.scalar.activation(t2[ni][: n1 - n0, :], ps[: n1 - n0, 0:SH], AF.Gelu,
                             bias=tb2_c[ni], scale=1.0)

    mm_proj(nc, pp, tw2_t, t1, 640, D, evict_t2)
    xs2 = [ap_.tile([128, SH], BF16, tag=f"xs2_{ci}", bufs=1, name=f"xs2_{ci}") for ci in range(3)]

    def evict_t3(ps, ni, n0, n1):
        nc.vector.scalar_tensor_tensor(out=xs2[ni][: n1 - n0, :], in0=ps[: n1 - n0, 0:SH],
                                       scalar=tb3_c[ni], in1=xs1[ni][: n1 - n0, :],
                                       op0=OP.add, op1=OP.add)

    mm_proj(nc, pp, tw3_t, t2, D, D, evict_t3)

    # ---- ff MLP + post-LN ----
    f1 = [tp.tile([128, SH], BF16, tag=f"f1_{ni}", bufs=1, name=f"f1_{ni}") for ni in range(10)]

    def evict_f1(ps, ni, n0, n1):
        nc.scalar.activation(f1[ni][: n1 - n0, :], ps[: n1 - n0, 0:SH], AF.Gelu,
                             bias=fb1_c[ni], scale=1.0)

    mm_proj(nc, pp, fw1_t, xs2, D, FFD, evict_f1)
    res2 = [tp.tile([128, SH], BF16, tag=f"res{ci}", bufs=2, name=f"res2_{ci}") for ci in range(3)]

    def evict_f2(ps, ni, n0, n1):
        nc.vector.scalar_tensor_tensor(out=res2[ni][: n1 - n0, :], in0=ps[: n1 - n0, 0:SH],
                                       scalar=fb2_c[ni], in1=xs2[ni][: n1 - n0, :],
                                       op0=OP.add, op1=OP.add)

    mm_proj(nc, pp, fw2_t, f1, FFD, D, evict_f2)
    return layernorm(nc, ap_, tp, pp, res2, l2g_c, l2b_c, ones, ones_f, eps_t, "xsb",
                     ones_row=True)


def layernorm(nc, ap_, tp, pp, res, g_c, b_c, ones, ones_f, eps_t, tag, ones_row=False):
    """LN over the feature (partition) dim of res (3 chunk tiles [kc, SH] bf16).
    Per-token stats are computed on [1, SH] rows, then broadcast to [128, SH]
    via two f32 matmuls for the partition-parallel normalize passes."""
    ps_s = ps_work(pp, "ps_s")
    ps_q = ps_work(pp, "ps_q")
    for ci, (c0, c1) in enumerate(DCH):
        kc = c1 - c0
        sq = tp.tile([128, SH], BF16, tag=f"sq{ci}", bufs=1, name=f"sq{ci}")
        nc.scalar.square(sq[:kc, :], res[ci][:kc, :])
        nc.tensor.matmul(ps_s[:1, 0:SH], ones[:kc, 0:1], res[ci][:kc, :],
                         start=(ci == 0), stop=(ci == 2))
        nc.tensor.matmul(ps_q[:1, 0:SH], ones[:kc, 0:1], sq[:kc, :],
                         start=(ci == 0), stop=(ci == 2))
    mrow = tp.tile([1, SH], F32, tag="m_row", bufs=1, name="mrow")
    nc.vector.tensor_scalar(out=mrow[:], in0=ps_s[:1, 0:SH], scalar1=1.0 / D, scalar2=None,
                            op0=OP.mult)
    m2 = tp.tile([1, SH], F32, tag="m2_row", bufs=1, name="m2")
    nc.vector.tensor_tensor(out=m2[:], in0=mrow[:], in1=mrow[:], op=OP.mult)
    varr = tp.tile([1, SH], F32, tag="var_row", bufs=1, name="varr")
    nc.vector.scalar_tensor_tensor(out=varr[:], in0=ps_q[:1, 0:SH], scalar=1.0 / D,
                                   in1=m2[:], op0=OP.mult, op1=OP.subtract)
    sd = tp.tile([1, SH], F32, tag="sd_row", bufs=1, name="sd")
    nc.scalar.activation(sd[:], varr[:], AF.Sqrt, bias=eps_t[0:1, 0:1])
    rstd = tp.tile([1, SH], F32, tag="rstd_row", bufs=1, name="rstd")
    nc.vector.reciprocal_approx_fast(rstd[:], sd[:])
    ps_bm = pp.tile([128, SH], F32, tag="ps_av0", bufs=1, name="ps_bm")
    nc.tensor.matmul(ps_bm[:], ones_f[0:1, :], mrow[:], start=True, stop=True)
    ps_br = pp.tile([128, SH], F32, tag="ps_av1", bufs=1, name="ps_br")
    nc.tensor.matmul(ps_br[:], ones_f[0:1, :], rstd[:], start=True, stop=True)
    out = [ap_.tile([128, SH], BF16, tag=f"{tag}{ci}", bufs=1, name=f"{tag}{ci}") for ci in range(3)]
    for ci, (c0, c1) in enumerate(DCH):
        kc = c1 - c0
        tmp = tp.tile([128, SH], BF16, tag="lnt", bufs=1, name="lnt")
        nc.vector.tensor_tensor(out=tmp[:kc, :], in0=res[ci][:kc, :], in1=ps_bm[:kc, :],
                                op=OP.subtract)
        nc.vector.scalar_tensor_tensor(out=out[ci][:kc, :], in0=tmp[:kc, :], scalar=g_c[ci],
                                       in1=ps_br[:kc, :], op0=OP.mult, op1=OP.mult)
        nc.vector.tensor_scalar(out=out[ci][:kc, :], in0=out[ci][:kc, :], scalar1=b_c[ci],
                                scalar2=None, op0=OP.add)
    if ones_row:
        nc.vector.memset(out[2][64:65, :], 1.0)
    return out


def conv_head(nc, cp, wp, ap_, tp, pp, dram, P, xs, vec_aps, ones, ones_f):
    cb_t = [vec_aps(f"cb{kk_i}", D) for kk_i in range(4)]
    # prefetch the fusion + head weights now; they land during the conv blocks
    fch = []
    row = 0
    for kk_i in range(4):
        for ci, (c0, c1) in enumerate(DCH):
            fch.append((row, row + (c1 - c0)))
            row += c1 - c0
    fus_t = load_w(nc, wp, P["fusw"], None, FFD, D, "fusw", bufs=1, boundaries=fch)
    ow_t = {nm: load_w(nc, wp, P[nm], None, kd, nd, nm, bufs=1)
            for nm, kd, nd in (("ow1", D, 160), ("ow2", 160, 80), ("ow3", 80, 10))}
    cwg = {}

    def get_cw(tap_):
        g = tap_ // 2
        if g not in cwg:
            cwg.clear()
            cwg[g] = load_w(nc, wp, P["ck"], g, D, 2 * D, "cw", bufs=3)
        return cwg[g], D * (tap_ % 2)

    feats = []

    # 1x1 conv straight from the resident activations — no gather dependency;
    # runs while the pair AllGather + pad construction are in flight
    cw0, coff0 = get_cw(0)
    ps_c1 = [None] * 3
    for ni, (n0, n1) in enumerate(DCH):
        ps_c1[ni] = ps_work(pp, "ps_c1")
        for ci, (c0, c1) in enumerate(DCH):
            nc.tensor.matmul(ps_c1[ni][: n1 - n0, 0:SH], cw0[ci][: c1 - c0, coff0 + n0:coff0 + n1],
                             xs[ci][: c1 - c0, :], start=(ci == 0), stop=(ci == 2))
    for ni, (n0, n1) in enumerate(DCH):
        ft = ap_.tile([128, SH], BF16, tag=f"ft0_{ni}", bufs=1, name=f"ft0_{ni}")
        nc.scalar.activation(ft[: n1 - n0, :], ps_c1[ni][: n1 - n0, 0:SH], AF.Relu,
                             bias=cb_t[0][ni], scale=1.0)
        feats.append(ft)
    ps_wk = ps_work(pp, "ps_wk")
    warm_keep(nc, ones, ps_wk[0:1, 0:SH], 72)

    # gather final xs across the pair
    xin = dram.tile([D, SH], BF16, tag="xin", bufs=1, name="xin")
    xout = dram.tile([2, D, SH], BF16, tag="xout", bufs=1, name="xout")
    for ci, (c0, c1) in enumerate(DCH):
        nc.sync.dma_start(out=xin[c0:c1, :], in_=xs[ci][: c1 - c0, :])
    nc.gpsimd.collective_compute("AllGather", OP.bypass, replica_groups=PAIRS,
                                 ins=[xin[:].opt()], outs=[xout[:].opt()])
    # padded full grid + own 21x36 window, all in SBUF (one dynamic-offset DVE copy)
    off_e = {}
    for eng_ in (nc.vector, nc.gpsimd):
        off_e[eng_] = (eng_.partition_id() % 2) * (15 * PG)
    pad = [ap_.tile([128, 21 * PG], BF16, tag=f"pad{ci}", bufs=1, name=f"pad{ci}") for ci in range(3)]
    for ci, (c0, c1) in enumerate(DCH):
        kc = c1 - c0
        eng = nc.gpsimd if ci == 1 else nc.vector
        xfull = ap_.tile([128, S], BF16, tag="xfull", bufs=1, name="xfull")
        nc.sync.dma_start(out=xfull[:kc].rearrange("p (g q) -> p g q", g=2),
                          in_=xout[:, c0:c1, :].rearrange("g p q -> p g q"))
        xpadf = ap_.tile([128, PG * PG], BF16, tag=f"xpadf{ci % 2}", bufs=1, name="xpadf")
        eng.memset(xpadf[:kc], 0.0)
        eng.tensor_copy(xpadf[:kc].rearrange("p (r c) -> p r c", r=PG)[:, 3:3 + G, 3:3 + G],
                        xfull[:kc].rearrange("p (r c) -> p r c", r=G))
        eng.tensor_copy(pad[ci][:kc, :], xpadf[:kc, bass.ds(off_e[eng], 21 * PG)])

    tap = 1
    cv_tags = ["ps_av0", "ps_av1"]
    for kk_i, kk in enumerate((3, 5, 7)):
        r = kk // 2
        ntaps = kk * kk
        ps_cv = [pp.tile([128, SH], F32, tag=cv_tags[0], bufs=1, name="ps_cv0"),
                 pp.tile([128, SH], F32, tag=cv_tags[1], bufs=1, name="ps_cv1"),
                 ps_work(pp, "ps_cv2")]
        for ti in range(ntaps):
            dy, dx = ti // kk - r, ti % kk - r
            cw, coff = get_cw(tap)
            tap += 1
            # contiguous shifted copies: strided-rhs matmuls never reach the
            # 2.4 GHz p-state; DVE assembles [128, SH] tiles the PE can stream
            rsh = []
            for ci, (c0, c1) in enumerate(DCH):
                t_ = tp.tile([128, SH], BF16, tag=f"rsh{ci}", bufs=3, name=f"rsh{ci}")
                eng = nc.gpsimd if ci == 2 else nc.vector
                eng.tensor_copy(
                    t_[: c1 - c0, :].rearrange("p (r c) -> p r c", r=15),
                    pad[ci][: c1 - c0, :].rearrange("p (r c) -> p r c", r=21)[
                        :, 3 + dy:18 + dy, 3 + dx:3 + dx + G])
                rsh.append(t_)
            for ni, (n0, n1) in enumerate(DCH):
                out_ps = ps_cv[ni] if ni < 2 else ps_cv[2]
                for ci, (c0, c1) in enumerate(DCH):
                    nc.tensor.matmul(out_ps[: n1 - n0, 0:SH], cw[ci][: c1 - c0, coff + n0:coff + n1],
                                     rsh[ci][: c1 - c0, :], start=(ti == 0 and ci == 0),
                                     stop=(ti == ntaps - 1 and ci == 2))
        for ni, (n0, n1) in enumerate(DCH):
            out_ps = ps_cv[ni] if ni < 2 else ps_cv[2]
            ft = ap_.tile([128, SH], BF16, tag=f"ft{kk_i + 1}_{ni}", bufs=1, name=f"ft{kk_i + 1}_{ni}")
            nc.scalar.activation(ft[: n1 - n0, :], out_ps[: n1 - n0, 0:SH], AF.Relu,
                                 bias=cb_t[kk_i + 1][ni], scale=1.0)
            feats.append(ft)

    # fus: [1280 -> 320], contraction chunks follow the feat tile boundaries
    fusb_c = vec_aps("fusb", D)
    fused = [tp.tile([128, SH], BF16, tag=f"fused{ni}", bufs=1, name=f"fused{ni}") for ni in range(3)]
    for ni, (n0, n1) in enumerate(DCH):
        ps = ps_work(pp, "ps_fus")
        for ci, (r0, r1) in enumerate(fch):
            nc.tensor.matmul(ps[: n1 - n0, 0:SH], fus_t[ci][: r1 - r0, n0:n1], feats[ci][: r1 - r0, :],
                             start=(ci == 0), stop=(ci == len(fch) - 1))
        nc.vector.tensor_scalar(out=fused[ni][: n1 - n0, :], in0=ps[: n1 - n0, 0:SH],
                                scalar1=fusb_c[ni], scalar2=None, op0=OP.add)

    # output head
    def head_mm(X, wname, bname, kdim, ndim, gelu, name, out_dt=BF16):
        wt = ow_t[wname]
        bt = vec_aps(bname, ndim)
        outs = [tp.tile([128, SH], out_dt, tag=f"{name}o{ni}", bufs=1, name=f"{name}o{ni}")
                for ni in range(len(chunks(ndim)))]

        def ev(ps, ni, n0, n1):
            if gelu:
                nc.scalar.activation(outs[ni][: n1 - n0, :], ps[: n1 - n0, 0:SH],
                                     AF.Gelu, bias=bt[ni], scale=1.0)
            else:
                nc.vector.tensor_scalar(out=outs[ni][: n1 - n0, :], in0=ps[: n1 - n0, 0:SH],
                                        scalar1=bt[ni], scalar2=None, op0=OP.add)

        mm_proj(nc, pp, wt, X, kdim, ndim, ev)
        return outs

    h1 = head_mm(fused, "ow1", "ob1", D, 160, True, "ow1")
    h2 = head_mm(h1, "ow2", "ob2", 160, 80, True, "ow2")
    lg = head_mm(h2, "ow3", "ob3", 80, 10, False, "ow3", out_dt=F32)  # [10, SH] f32

    nc.sync.dma_start(out=P["out"][:], in_=lg[0][:10, :])


# ======================= host side =======================

def prep_inputs(inputs):
    """Full inputs -> list of 8 per-core input dicts."""
    import ml_dtypes
    bf16 = ml_dtypes.bfloat16
    f32 = np.float32
    ip = {k: np.asarray(v) for k, v in inputs.items()}

    def bf(x):
        return np.ascontiguousarray(np.asarray(x, f32)).astype(bf16)

    com = {}
    com["iota10"] = np.arange(10, dtype=f32).reshape(10, 1)
    com["inw"] = bf(ip["in_emb_w"])

    # head-padded q/k/v/o layouts (64-wide slot per head; v has the sum slot at 64h)
    wqp = np.zeros((L, D, HP), f32)
    wkp = np.zeros((L, D, HP), f32)
    wvp = np.zeros((L, D, HP), f32)
    wop = np.zeros((L, HP, D), f32)
    bqp = np.zeros((L, HP), f32)
    bkp = np.zeros((L, HP), f32)
    bvp = np.zeros((L, HP), f32)
    for h in range(NH):
        hs = slice(40 * h, 40 * h + 40)
        wqp[:, :, 64 * h:64 * h + 40] = ip["wq"][:, :, hs]
        wkp[:, :, 64 * h:64 * h + 40] = ip["wk"][:, :, hs]
        wvp[:, :, 64 * h + 1:64 * h + 41] = ip["wv"][:, :, hs]
        wop[:, 64 * h + 1:64 * h + 41, :] = ip["wo"][:, hs, :]
        bqp[:, 64 * h:64 * h + 40] = ip["bq"][:, hs] * SCALE
        bkp[:, 64 * h:64 * h + 40] = ip["bk"][:, hs]
        bvp[:, 64 * h] = 1.0
        bvp[:, 64 * h + 1:64 * h + 41] = ip["bv"][:, hs]
    wsrc = {"wq": wqp, "wk": wkp, "wv": wvp, "wo": wop}
    wsrc["wcat"] = np.concatenate([ip["w_rot"], ip["w_refl"], ip["w_tr"], ip["w_sc"]],
                                  axis=2) * (1.0 / S)
    wsrc["tw3"] = ip["tn_w3"] * 0.3
    wsrc["fw1"] = ip["ff_w1"]
    com["bcat"] = bf(np.concatenate([ip["b_rot"], ip["b_refl"], ip["b_tr"], ip["b_sc"]],
                                    axis=1).reshape(L, 1, 17))
    mk = np.zeros((17, 4), f32)
    mk[0:4, 0] = 1.0
    mk[4:12, 1] = 1.0
    mk[14:17, 2] = 1.0
    mk[12:14, 3] = 1.0
    com["mskp"] = bf(mk)
    mkT = np.zeros((3, 17), f32)
    mkT[0, 0:4] = 1.0
    mkT[1, 4:12] = 1.0
    mkT[2, 14:17] = 1.0
    com["mskTp"] = bf(mkT)
    wsrc["tw1"] = ip["tn_w1"][:, :D, :]
    wsrc["tw1t"] = np.ascontiguousarray(ip["tn_w1"][:, D:D + 17, :])
    wsrc["tw2"] = ip["tn_w2"]
    wsrc["fw2"] = ip["ff_w2"]

    def pack_w(specs, cm, ncols):
        pk = np.zeros((L, 128, ncols), f32)
        for name, kdim, ndim, bnd in specs:
            arr = np.asarray(wsrc[name], f32)
            for ci, (c0, c1) in enumerate(bnd or chunks(kdim)):
                off = cm[(name, ci)]
                pk[:, : c1 - c0, off:off + ndim] = arr[:, c0:c1, :]
        return pk

    pkA = pack_w(WPK_A, WPKA_CM, WPKA_COLS)
    # v bias + softmax-sum one-slots ride contraction row 64 of the wv chunk-2 tile
    offv = WPKA_CM[("wv", 2)]
    pkA[:, 64, offv:offv + HP] = bvp
    com["wpackA"] = bf(pkA)
    com["wpackM"] = bf(pack_w(WPK_M, WPKM_CM, WPKM_COLS))
    taps = np.concatenate([ip["ck1"].reshape(1, D, D), ip["ck3"].reshape(9, D, D),
                           ip["ck5"].reshape(25, D, D), ip["ck7"].reshape(49, D, D)], axis=0)
    com["ck"] = bf(taps.reshape(NTAPG, 2, D, D).transpose(0, 2, 1, 3).reshape(NTAPG, D, 2 * D))
    com["fusw"] = bf(ip["fus_w"])
    com["ow1"], com["ow2"], com["ow3"] = bf(ip["op_w1"]), bf(ip["op_w2"]), bf(ip["op_w3"])

    # packed per-partition vectors
    vec_src = {}
    for l in range(L):
        vec_src[("bqs", l)] = bqp[l]
        vec_src[("bk", l)] = bkp[l]
        vec_src[("bo", l)] = ip["bo"][l]
        vec_src[("lag", l)] = ip["ln_a_g"][l]
        vec_src[("lab", l)] = ip["ln_a_b"][l]
        vec_src[("l2g", l)] = ip["ln2_g"][l]
        vec_src[("l2b", l)] = ip["ln2_b"][l]
        vec_src[("tb1", l)] = ip["tn_b1"][l]
        vec_src[("tb2", l)] = ip["tn_b2"][l]
        vec_src[("tb3", l)] = ip["tn_b3"][l] * 0.3
        vec_src[("fb1", l)] = ip["ff_b1"][l]
        vec_src[("fb2", l)] = ip["ff_b2"][l]
    vec_src[("inb", None)] = ip["in_emb_b"]
    for i, kk in enumerate((1, 3, 5, 7)):
        vec_src[(f"cb{i}", None)] = ip[f"cb{kk}"]
    vec_src[("fusb", None)] = ip["fus_b"]
    vec_src[("ob1", None)] = ip["op_b1"]
    vec_src[("ob2", None)] = ip["op_b2"]
    vec_src[("ob3", None)] = ip["op_b3"]
    vecpack = np.zeros((128, VEC_NCOL), f32)
    for (name, l, ci), col in VEC_COLMAP.items():
        src = np.asarray(vec_src[(name, l)], f32)
        c0, c1 = chunks(len(src))[ci]
        vecpack[: c1 - c0, col] = src[c0:c1]
    com["vecpack"] = vecpack

    # geometric bias, transposed + kchunk-padded + head-pair interleaved:
    # [l, kchunk, g2, r, 2*q]; per core, key rows are reordered [own | peer]
    dist_idx, dir_idx = ip["dist_idx"], ip["dir_idx"]
    bhkq_l = []
    for l in range(L):
        bqk = ip["dist_emb"][l][dist_idx] + ip["dir_emb"][l][dir_idx]   # [q, k, h] f32
        bhkq_l.append(np.ascontiguousarray(bqk.transpose(2, 1, 0)) * BIAS_SCALE)  # [h, k, q]
    bias_half = []
    f8 = None
    import ml_dtypes as _md
    f8 = _md.float8_e4m3fn
    for half in range(2):
        own = slice(SH * half, SH * half + SH)
        peer_s = slice(SH * (1 - half), SH * (1 - half) + SH)
        bt = np.zeros((L, len(KCH), 128, NH // 2, 2 * SH), dtype=f8)
        for l in range(L):
            ordered = np.concatenate([bhkq_l[l][:, own, own], bhkq_l[l][:, peer_s, own]], axis=1)
            for ci, (k0, k1) in enumerate(KCH):
                for g2 in range(NH // 2):
                    bt[l, ci, : k1 - k0, g2, 0:SH] = ordered[2 * g2, k0:k1, :].astype(f8)
                    bt[l, ci, : k1 - k0, g2, SH:2 * SH] = ordered[2 * g2 + 1, k0:k1, :].astype(f8)
        bias_half.append(bt)

    peT_full = np.ascontiguousarray(ip["pe"].reshape(S, D).T.astype(f32))  # [D, S]
    grids = ip["input_grid"].reshape(B, S)

    in_maps = []
    for c in range(8):
        b, half = c // 2, c % 2
        t0, t1 = SH * half, SH * (1 - half)
        m = dict(com)
        m["grid"] = np.concatenate([grids[b, t0:t0 + SH], grids[b, t1:t1 + SH]]
                                   ).astype(f32).reshape(1, S).astype(bf16)
        m["peT"] = bf(np.concatenate([peT_full[:, t0:t0 + SH], peT_full[:, t1:t1 + SH]], axis=1))
        m["biasT"] = bias_half[half]
        in_maps.append(m)
    return in_maps


_BUILT = None


def _fuse_ldweights(nc):
    """Drop tile_legalize's explicit InstLdweights (the paired InstMatmult is
    still self-loading); keep their sync waits/updates on EventSemaphores so
    walrus can compile with --enable-ldw-opt=true and background the loads."""
    for f in nc.m.functions:
        for bb in f.blocks:
            il = bb.instructions
            newlist = []
            changed = False
            for i, ins in enumerate(il):
                if type(ins).__name__ == "InstLdweights":
                    changed = True
                    if i + 1 < len(il) and type(il[i + 1]).__name__ == "InstMatmult":
                        il[i + 1].ldweights = True   # matmul self-loads now
                    si = ins.sync_info
                    nw = len(si.on_wait) if si else 0
                    nu = len(si.on_update) if si else 0
                    if nw == 0 and nu == 0:
                        continue
                    ev = mybir.InstEventSemaphore(
                        name=f"ldwev_{ins.name}", engine=ins.engine,
                        ins=[], outs=[], sync_info=si, debug=ins.debug)
                    newlist.append(ev)
                    continue
                newlist.append(ins)
            if changed:
                bb.instructions = newlist


def get_built():
    global _BUILT
    if _BUILT is None:
        import os
        nc = bacc.Bacc("TRN2", target_bir_lowering=False, num_devices=8)
        build(nc)
        nc.finalize()
        if os.environ.get("ATLAS_LDWFUSE") == "1":
            _fuse_ldweights(nc)
        _BUILT = nc
    return _BUILT


_LDW_PATCHED = False


def _enable_ldw_opt():
    """Compile NEFFs with --enable-ldw-opt=true (overlaps LDWEIGHTS with matmuls)."""
    global _LDW_PATCHED
    if _LDW_PATCHED:
        return
    import concourse.bass_utils as bu
    orig = bu.run_command

    def patched(cmd, cwd=None, **kw):
        cmd = ["--enable-ldw-opt=true" if c == "--enable-ldw-opt=false" else c for c in cmd]
        return orig(cmd, cwd=cwd, **kw)

    bu.run_command = patched
    _LDW_PATCHED = True


def kernel(**inputs):
    from concourse.bass_utils import run_bass_kernel_spmd
    import os
    if os.environ.get("ATLAS_LDWOPT") == "1":
        _enable_ldw_opt()
    nc = get_built()
    in_maps = prep_inputs(inputs)
    trace = bool(os.environ.get("ATLAS_TRACE"))
    res = run_bass_kernel_spmd(nc, in_maps, core_ids=list(range(8)), trace=trace)
    if trace:
        kernel.last_exec_time_ns = res.exec_time_ns
        kernel.last_results = res
    out = np.zeros((B, G, G, 10), np.float32)
    for c in range(8):
        b, half = c // 2, c % 2
        out[b, 15 * half:15 * half + 15] = res.results[c]["out"].T.reshape(15, G, 10)
    return out
